# revision 30
# baseline (speedup 1.0000x reference)
"""Trainium2 Bass kernel for nn_BaseGenerator (4-layer dense transformer).

Strategy: pure data-parallel over batch (B=8 -> 8 NeuronCores, no
collectives).  Each core runs the full transformer on one batch element.
Activations are kept feature-major [E, S] in bf16 so every GEMM contracts
over the partition dim; PSUM accumulates in fp32.

Key performance structure:
  - exp(mask) shipped from host, resident in SBUF for all 4 layers;
    applied MULTIPLICATIVELY on the Vector engine (at = exp(s) * expmask),
    so no PE matmuls or per-layer DMA for masking.
  - causal truncation: score/AV matmuls + exp only cover q >= kc*128 for
    key chunk kc (62.5% of full work).
  - Wo contracts K=128 over paired heads (ctx stored as [128, S] pairs).
  - All plain PSUM->SBUF copies/bias/residual ops run on Vector (DVE);
    Scalar engine only runs Exp / Gelu / Sqrt, avoiding activation-table
    thrashing.
  - token embeddings (val+ring gather) precomputed on host.
  - zero biases / unit LN scales detected host-side and elided.
"""

import os
import sys

for _p in ("/opt/trn_rl_repo",):
    if _p not in sys.path:
        sys.path.insert(0, _p)

import ml_dtypes
import numpy as np

import concourse.bass as bass
import concourse.mybir as mybir
import concourse.tile as tile
from concourse import bacc
from concourse.bass_utils import run_bass_kernel_spmd

BF16 = ml_dtypes.bfloat16

L, E, H, F = 4, 1024, 16, 4096
B, S = 8, 512
VV, VR = 40, 30
DIST_V = 200
PAD_ID = 0
DH = E // H  # 64
NE = E // 128  # 8 feature chunks
NO = 10  # logit row tiles (1280 padded)
EM_OFF = (0, 512, 896, 1152)  # causal-packed expmask offsets per kc
EM_W = 1280
FULLW = False

f32 = mybir.dt.float32
bf16 = mybir.dt.bfloat16
AF = mybir.ActivationFunctionType
OP = mybir.AluOpType

_CACHE = {}


# ----------------------------------------------------------------------------
# host-side input prep
# ----------------------------------------------------------------------------

def _b16(x):
    return np.ascontiguousarray(np.asarray(x).astype(BF16))


def _block_lhsT(W, gsize):
    # W: [L?, OUT, IN] -> [.., G, 128, IN//128, gsize] with
    # out[..., g, p, c, o] = W[..., g*gsize + o, c*128 + p]
    *lead, O, I = W.shape
    G = O // gsize
    nc_ = I // 128
    Wb = W.reshape(*lead, G, gsize, nc_, 128)
    Wb = np.moveaxis(Wb, -1, -3)  # [..., G, 128, gsize, nc]
    Wb = np.swapaxes(Wb, -1, -2)  # [..., G, 128, nc, gsize]
    return np.ascontiguousarray(Wb)


def _pp(v):  # [..., N*128] -> [..., 128, N]
    *lead, N = v.shape
    return np.ascontiguousarray(
        v.reshape(*lead, N // 128, 128).swapaxes(-1, -2).astype(np.float32)
    )


def _prep_shared(inp):
    """Weight-layout prep shared by all cores. Returns (tensors, flags)."""
    out = {}
    flags = {}

    Wqkv = np.asarray(inp["Wqkv"], np.float32).copy()  # [L, 3E, E]
    bqkv = np.asarray(inp["bqkv"], np.float32).copy()  # [L, 3E]
    # fold attention scale into Q projection
    scale = 1.0 / np.sqrt(DH)
    Wqkv[:, :E, :] *= scale
    bqkv[:, :E] *= scale

    out["wqkv"] = _b16(_block_lhsT(Wqkv, 512))          # [L, 6, 128, 8, 512]

    # Wo paired-head lhsT: wo[l, p, hp, mi, m] = Wo[l, mi*128+m, hp*128+p]
    Wo = np.asarray(inp["Wo"], np.float32)  # [L, E(out), E(in=ctx)]
    wo = Wo.reshape(L, 8, 128, 8, 128)      # [l, mi, m, hp, p]
    wo = wo.transpose(0, 4, 3, 1, 2)        # [l, p, hp, mi, m]
    out["wo"] = _b16(wo)                    # [L, 128, 8, 8, 128]

    out["w1"] = _b16(_block_lhsT(np.asarray(inp["W1"], np.float32), 512))
    W2 = np.asarray(inp["W2"], np.float32)  # out=E, in=F
    w2b = _block_lhsT(W2, 512)  # [L, 2, 128, 32, 512]
    w2b = w2b.reshape(L, 2, 128, 4, 8, 512).transpose(0, 1, 3, 2, 4, 5)
    out["w2"] = _b16(w2b)  # [L, 2, 4, 128, 8, 512]

    genW = np.asarray(inp["gen_W"], np.float32)  # [1200, E]
    genW_pad = np.zeros((1280, E), np.float32)
    genW_pad[:1200] = genW
    genw_b = _block_lhsT(genW_pad, 640).reshape(2, 128, 2, 4, 640)
    genw_b = genw_b.transpose(0, 2, 1, 3, 4).reshape(4, 128, 4, 640)
    out["genw"] = _b16(genw_b)  # [4, 128, 4, 640]

    # biases (elided when zero)
    flags["bqkv"] = bool(np.any(bqkv != 0))
    flags["bo"] = bool(np.any(np.asarray(inp["bo"]) != 0))
    flags["b1"] = bool(np.any(np.asarray(inp["b1"]) != 0))
    flags["b2"] = bool(np.any(np.asarray(inp["b2"]) != 0))
    flags["genb"] = bool(np.any(np.asarray(inp["gen_b"]) != 0))
    flags["ln_s"] = not bool(
        np.all(np.asarray(inp["ln1_s"]) == 1) and np.all(np.asarray(inp["ln2_s"]) == 1)
    )
    flags["ln_b"] = bool(
        np.any(np.asarray(inp["ln1_b"]) != 0) or np.any(np.asarray(inp["ln2_b"]) != 0)
    )
    flags["lnf_s"] = not bool(np.all(np.asarray(inp["lnf_s"]) == 1))
    flags["lnf_b"] = bool(np.any(np.asarray(inp["lnf_b"]) != 0))

    if flags["bqkv"]:
        out["bqkv_pp"] = _pp(bqkv[:, : 2 * E])  # [L, 128, 16] (Q scaled)
        out["bv_row"] = _b16(bqkv[:, 2 * E:].reshape(L, 1, E))  # [L, 1, E]
        out["ones_row"] = _b16(np.ones((1, S), np.float32))
    if flags["bo"]:
        out["bo_pp"] = _pp(np.asarray(inp["bo"], np.float32))  # [L, 128, 8]
    if flags["b1"]:
        out["b1_pp"] = _pp(np.asarray(inp["b1"], np.float32))  # [L, 128, 32]
    if flags["b2"]:
        out["b2_pp"] = _pp(np.asarray(inp["b2"], np.float32))  # [L, 128, 8]
    if flags["genb"]:
        gbp = np.zeros((1280,), np.float32)
        gbp[:1200] = np.asarray(inp["gen_b"], np.float32)
        out["gen_b_pp"] = np.ascontiguousarray(gbp.reshape(NO, 128).T)  # [128, 10]
    if flags["ln_s"]:
        ln_s = np.stack([np.asarray(inp["ln1_s"], np.float32),
                         np.asarray(inp["ln2_s"], np.float32)], 1)  # [L, 2, E]
        out["ln_s_pp"] = _pp(ln_s)  # [L, 2, 128, 8]
    if flags["ln_b"]:
        ln_b = np.stack([np.asarray(inp["ln1_b"], np.float32),
                         np.asarray(inp["ln2_b"], np.float32)], 1)
        out["ln_b_pp"] = _pp(ln_b)
    if flags["lnf_s"]:
        out["lnf_s_pp"] = _pp(np.asarray(inp["lnf_s"], np.float32))  # [128, 8]
    if flags["lnf_b"]:
        out["lnf_b_pp"] = _pp(np.asarray(inp["lnf_b"], np.float32))

    out["ones_col"] = _b16(np.ones((128, 1), np.float32))
    out["ones_r128"] = _b16(np.ones((1, 128), np.float32))
    return out, flags


def _prep_percore(inp):
    """Per-core tensors: embedded tokens + packed exp(mask)."""
    val = np.asarray(inp["val_sequences"]).astype(np.int64)    # [B, S]
    ring = np.asarray(inp["ring_sequences"]).astype(np.int64)  # [B, S]
    dist = np.asarray(inp["distance_squares"]).astype(np.int64)  # [B, S, S]
    de = np.asarray(inp["dist_emb"], np.float32)  # [200, H]
    ve = np.asarray(inp["val_emb"], np.float32)   # [VV, E]
    re = np.asarray(inp["ring_emb"], np.float32)  # [VR, E]

    # x0: feature-major embedded tokens [B, NE, 128, S]
    x0 = (ve[val] + re[ring]) * np.sqrt(E)                  # [B, S, E]
    x0 = x0.reshape(B, S, NE, 128).transpose(0, 2, 3, 1)    # [B, NE, 128, S]
    x0 = _b16(x0)

    # expmask, causal-packed: em[b, p, h*1280 + OFF[kc] + j]
    #   = exp(de[dist[b, qlo+j, kc*128+p]])  (0 where k>q or key is pad)
    g = np.exp(de[dist])                 # [B, q, k, H]
    kk = np.arange(S)
    keep = (kk[None, :] <= kk[:, None])  # [q, k]: k <= q
    g *= keep[None, :, :, None]
    g *= (val != PAD_ID)[:, None, :, None]
    emp = np.zeros((B, H, 128, EM_W), np.float32)
    for kc in range(4):
        qlo = kc * 128
        w = S - qlo
        blk = g[:, qlo:, kc * 128:(kc + 1) * 128, :]  # [B, w, 128, H]
        emp[:, :, :, EM_OFF[kc]:EM_OFF[kc] + w] = blk.transpose(0, 3, 2, 1)
    emp = _b16(emp.transpose(0, 2, 1, 3).reshape(B, 128, H * EM_W))

    cores = []
    for b in range(B):
        cores.append({"x0": np.ascontiguousarray(x0[b]),
                      "emask": np.ascontiguousarray(emp[b])})
    return cores


# ----------------------------------------------------------------------------
# device program
# ----------------------------------------------------------------------------

def _declare(nc, flags):
    d = {}

    def di(name, shape, dt):
        d[name] = nc.dram_tensor(name, list(shape), dt, kind="ExternalInput").ap()

    di("wqkv", (L, 6, 128, 8, 512), bf16)
    di("wo", (L, 128, 8, 8, 128), bf16)
    di("w1", (L, 8, 128, 8, 512), bf16)
    di("w2", (L, 2, 4, 128, 8, 512), bf16)
    di("genw", (4, 128, 4, 640), bf16)
    if flags["bqkv"]:
        di("bqkv_pp", (L, 128, 16), f32)
        di("bv_row", (L, 1, E), bf16)
        di("ones_row", (1, S), bf16)
    if flags["bo"]:
        di("bo_pp", (L, 128, 8), f32)
    if flags["b1"]:
        di("b1_pp", (L, 128, 32), f32)
    if flags["b2"]:
        di("b2_pp", (L, 128, 8), f32)
    if flags["genb"]:
        di("gen_b_pp", (128, NO), f32)
    if flags["ln_s"]:
        di("ln_s_pp", (L, 2, 128, 8), f32)
    if flags["ln_b"]:
        di("ln_b_pp", (L, 2, 128, 8), f32)
    if flags["lnf_s"]:
        di("lnf_s_pp", (128, 8), f32)
    if flags["lnf_b"]:
        di("lnf_b_pp", (128, 8), f32)
    di("ones_col", (128, 1), bf16)
    di("ones_r128", (1, 128), bf16)
    di("x0", (NE, 128, S), bf16)
    di("emask", (128, H * EM_W), bf16)
    d["logits"] = nc.dram_tensor(
        "logits", [NO, 128, S], f32, kind="ExternalOutput"
    ).ap()
    if os.environ.get("BG_DEBUG"):
        def do(name, shape):
            d[name] = nc.dram_tensor(name, list(shape), bf16,
                                     kind="ExternalOutput").ap()
        do("dbg_h0", (NE, 128, S))
        do("dbg_qk", (16, 128, S))
        do("dbg_v", (4, 128, H, DH + 1))
        do("dbg_at", (8, 128, S))
        do("dbg_ctx", (8, 128, S))
        do("dbg_r1", (NE, 128, S))
        do("dbg_h1", (NE, 128, S))
        do("dbg_h2", (NE, 128, S))
    return d


def _emit(nc, tc, d, ctx, flags):
    mm = nc.tensor.matmul

    cpool = ctx.enter_context(tc.tile_pool(name="cpool", bufs=1))
    wpool = ctx.enter_context(tc.tile_pool(name="wpool", bufs=4))
    hpool = ctx.enter_context(tc.tile_pool(name="hpool", bufs=16))
    qkpool = ctx.enter_context(tc.tile_pool(name="qkpool", bufs=17))
    vpool = ctx.enter_context(tc.tile_pool(name="vpool", bufs=4))
    atpool = ctx.enter_context(tc.tile_pool(name="atpool", bufs=5))
    ctxpool = ctx.enter_context(tc.tile_pool(name="ctxpool", bufs=8))
    ffpool = ctx.enter_context(tc.tile_pool(name="ffpool", bufs=32))
    tmppool = ctx.enter_context(tc.tile_pool(name="tmppool", bufs=4))
    smallf = ctx.enter_context(tc.tile_pool(name="smallf", bufs=4))
    smallb = ctx.enter_context(tc.tile_pool(name="smallb", bufs=2))
    recpool = ctx.enter_context(tc.tile_pool(name="recpool", bufs=2))
    outpool = ctx.enter_context(tc.tile_pool(name="outpool", bufs=2))
    pppool = ctx.enter_context(tc.tile_pool(name="pppool", bufs=4))
    empool = ctx.enter_context(tc.tile_pool(name="empool", bufs=1))

    ps_gemm = ctx.enter_context(tc.tile_pool(name="ps_gemm", bufs=4, space="PSUM"))
    ps_ctx = ctx.enter_context(tc.tile_pool(name="ps_ctx", bufs=3, space="PSUM"))
    ps_ln = ctx.enter_context(tc.tile_pool(name="ps_ln", bufs=1, space="PSUM"))

    hw = nc.sync  # HWDGE dma engine

    # --- constants -----------------------------------------------------------
    ones_col = cpool.tile([128, 1], bf16)
    hw.dma_start(out=ones_col, in_=d["ones_col"])
    ones_r128 = cpool.tile([1, 128], bf16)
    hw.dma_start(out=ones_r128, in_=d["ones_r128"])
    eps_t = cpool.tile([128, 1], f32)
    nc.vector.memset(eps_t, 1e-5)
    genb_pp = None
    if flags["genb"]:
        genb_pp = cpool.tile([128, NO], f32)
        hw.dma_start(out=genb_pp, in_=d["gen_b_pp"])
    lnf_s = lnf_b = None
    if flags["lnf_s"]:
        lnf_s = cpool.tile([128, 8], f32)
        hw.dma_start(out=lnf_s, in_=d["lnf_s_pp"])
    if flags["lnf_b"]:
        lnf_b = cpool.tile([128, 8], f32)
        hw.dma_start(out=lnf_b, in_=d["lnf_b_pp"])
    ones_row = bv_all = None
    if flags["bqkv"]:
        ones_row = cpool.tile([1, S], bf16)
        hw.dma_start(out=ones_row, in_=d["ones_row"])

    # resident expmask (on SWDGE queue so it doesn't delay weight loads)
    em_sb = empool.tile([128, H * EM_W], bf16)
    nc.gpsimd.dma_start(out=em_sb, in_=d["emask"])

    # --- embedding (host-precomputed) ---------------------------------------
    h_t = []
    for c in range(NE):
        ht = hpool.tile([128, S], bf16, tag="h")
        hw.dma_start(out=ht, in_=d["x0"][c])
        if "dbg_h0" in d:
            nc.sync.dma_start(out=d["dbg_h0"][c], in_=ht)
        h_t.append(ht)

    env = dict(locals())
    env["ps_ln"] = ps_ln

    # --- layers --------------------------------------------------------------
    # When ln2 has unit scale / zero bias, its output is exactly normalized
    # (mean 0, var 1-eps'), so a unit-scale/zero-bias final LN is an identity
    # up to O(eps) -- skip it.
    skip_lnf = not (flags["lnf_s"] or flags["lnf_b"]
                    or flags["ln_s"] or flags["ln_b"])
    lnacc_f = [None]
    for l in range(L):
        h_t = _layer(nc, tc, d, l, h_t, env, flags,
                     lnacc_f if (l == L - 1 and not skip_lnf) else None)

    # --- final LN + head -----------------------------------------------------
    with nc.named_scope("final"):
        if skip_lnf:
            hf = h_t
        else:
            rstdR, uR = lnacc_f[0].stats()
            hf = _ln_normalize(nc, env, h_t, rstdR, uR, lnf_s, lnf_b)
        genw_sb = env["genw_sb"]
        for mt in range(NO):
            g, mi = divmod(mt, 5)
            ps = ps_gemm.tile([128, S], f32, tag="gemm")
            for c in range(NE):
                mm(ps, genw_sb[g * 2 + c // 4][:, c % 4, mi * 128:(mi + 1) * 128],
                   hf[c],
                   start=(c == 0), stop=(c == NE - 1))
            ot = outpool.tile([128, S], f32, tag="f32out")
            if flags["genb"]:
                nc.vector.tensor_scalar(ot, ps, genb_pp[:, mt:mt + 1], None, OP.add)
            else:
                nc.vector.tensor_copy(ot, ps)
            hw.dma_start(out=d["logits"][mt], in_=ot)


class _LNAcc:
    """Pipelined LN statistics: sum/sumsq matmuls emitted as chunks land."""

    def __init__(self, nc, env, name, pool=None, tag="ln"):
        self.nc = nc
        self.env = env
        self.name = name
        pool = pool if pool is not None else env["ps_ln"]
        sums = pool.tile([33, S], f32, tag=tag, name=f"lns_{name}")
        self.sums_r = sums[0:1, :]
        self.sums_q = sums[32:33, :]

    def add(self, c, r_tile):
        nc, env = self.nc, self.env
        sq = env["tmppool"].tile([128, S], bf16, tag="sq",
                                 name=f"sq_{self.name}_{c}")
        nc.scalar.activation(sq, r_tile, AF.Square)
        mm = nc.tensor.matmul
        mm(self.sums_r, env["ones_col"], r_tile, start=(c == 0),
           stop=(c == NE - 1))
        mm(self.sums_q, env["ones_col"], sq, start=(c == 0),
           stop=(c == NE - 1))

    def stats(self):
        nc, env = self.nc, self.env
        smallf = env["smallf"]; smallb = env["smallb"]; recpool = env["recpool"]
        nm = self.name
        s2 = smallf.tile([1, S], f32, tag="sf", name=f"s2_{nm}")
        nc.scalar.activation(s2, self.sums_r, AF.Square)
        varE = smallf.tile([1, S], f32, tag="sf", name=f"ve_{nm}")
        # varE = sumsq - s2/E  (= E * var)
        nc.vector.scalar_tensor_tensor(varE, s2, -1.0 / E, self.sums_q,
                                       OP.mult, OP.add)
        rstd = smallf.tile([1, S], f32, tag="sf", name=f"rs_{nm}")
        nc.scalar.activation(rstd, varE, AF.Abs_reciprocal_sqrt,
                             bias=env["eps_t"][:1, :], scale=1.0 / E)
        ru = smallb.tile([1, 2 * S], bf16, tag="sb", name=f"ru_{nm}")
        nc.vector.tensor_copy(ru[:, 0:S], rstd)
        # u*rstd = (sum/E) * rstd
        nc.vector.scalar_tensor_tensor(ru[:, S:2 * S], self.sums_r, 1.0 / E,
                                       rstd, OP.mult, OP.mult)
        mm = nc.tensor.matmul
        ps_ctx = env["ps_ctx"]
        rstdR = ps_ctx.tile([128, S], f32, tag="ctxps", name=f"bcr_{nm}")
        mm(rstdR, env["ones_r128"], ru[:, 0:S], start=True, stop=True)
        uR = ps_ctx.tile([128, S], f32, tag="ctxps", name=f"bcu_{nm}")
        mm(uR, env["ones_r128"], ru[:, S:2 * S], start=True, stop=True)
        return rstdR, uR


def _ln_normalize(nc, env, r_t, rstdR, uR, s_pp, b_pp, nxt=None):
    out_t = []
    for c in range(NE):
        t2 = env["tmppool"].tile([128, S], bf16, tag="tmp")
        nc.vector.tensor_sub(t2, r_t[c], uR)
        ht = env["hpool"].tile([128, S], bf16, tag="h")
        sc = s_pp[:, c:c + 1] if s_pp is not None else 1.0
        nc.vector.scalar_tensor_tensor(ht, t2, sc, rstdR, OP.mult, OP.mult)
        if b_pp is not None:
            nc.vector.tensor_scalar(ht, ht, b_pp[:, c:c + 1], None, OP.add)
        if nxt is not None:
            nxt.add(c, ht)
        out_t.append(ht)
    return out_t


def _layer(nc, tc, d, l, h_t, env, flags, lnacc_f=None):
    mm = nc.tensor.matmul
    hw = nc.sync
    wpool = env["wpool"]; hpool = env["hpool"]; qkpool = env["qkpool"]
    vpool = env["vpool"]; atpool = env["atpool"]
    ctxpool = env["ctxpool"]; ffpool = env["ffpool"]; tmppool = env["tmppool"]
    smallf = env["smallf"]; smallb = env["smallb"]; recpool = env["recpool"]
    pppool = env["pppool"]
    ps_gemm = env["ps_gemm"]; ps_ctx = env["ps_ctx"]
    ones_col = env["ones_col"]; em_sb = env["em_sb"]

    # per-layer small params
    bqkv_pp = bv_row = bo_pp = b1_pp = b2_pp = None
    ln_s = [None, None]
    ln_b = [None, None]
    if flags["bqkv"]:
        bqkv_pp = pppool.tile([128, 16], f32, tag="pp16")
        hw.dma_start(out=bqkv_pp, in_=d["bqkv_pp"][l])
        bv_row = pppool.tile([1, E], bf16, tag="bvrow", bufs=2)
        hw.dma_start(out=bv_row, in_=d["bv_row"][l])
    if flags["bo"]:
        bo_pp = pppool.tile([128, 8], f32, tag="pp8")
        hw.dma_start(out=bo_pp, in_=d["bo_pp"][l])
    if flags["b1"]:
        b1_pp = pppool.tile([128, 32], f32, tag="pp32")
        hw.dma_start(out=b1_pp, in_=d["b1_pp"][l])
    if flags["b2"]:
        b2_pp = pppool.tile([128, 8], f32, tag="pp8")
        hw.dma_start(out=b2_pp, in_=d["b2_pp"][l])
    if flags["ln_s"]:
        ln_s = [pppool.tile([128, 8], f32, tag="pp8", name=f"lns{l}_{i}")
                for i in range(2)]
        for i in range(2):
            hw.dma_start(out=ln_s[i], in_=d["ln_s_pp"][l, i])
    if flags["ln_b"]:
        ln_b = [pppool.tile([128, 8], f32, tag="pp8", name=f"lnb{l}_{i}")
                for i in range(2)]
        for i in range(2):
            hw.dma_start(out=ln_b[i], in_=d["ln_b_pp"][l, i])

    # --- QKV -----------------------------------------------------------------
    with nc.named_scope(f"L{l}_qkv"):
        qk_t = []  # 16 tiles: q 0..7 (2 heads each), k 8..15
        for g in range(4):  # Q, K feature-major
            wt = wpool.tile([128, 8, 512], bf16, tag="w")
            hw.dma_start(out=wt, in_=d["wqkv"][l, g])
            for mi in range(4):
                mt = g * 4 + mi
                ps = ps_gemm.tile([128, S], f32, tag="gemm")
                for c in range(NE):
                    mm(ps, wt[:, c, mi * 128:(mi + 1) * 128], h_t[c],
                       start=(c == 0), stop=(c == NE - 1))
                qk = qkpool.tile([128, S], bf16, tag="qk")
                if flags["bqkv"]:
                    nc.scalar.activation(qk, ps, AF.Identity,
                                         bias=bqkv_pp[:, mt:mt + 1])
                else:
                    nc.scalar.activation(qk, ps, AF.Copy)
                if l == 0 and "dbg_qk" in d:
                    hw.dma_start(out=d["dbg_qk"][mt], in_=qk)
                qk_t.append(qk)
        # V token-major, augmented with ones column
        v_t = []
        for n in range(4):
            vt = vpool.tile([128, H, DH + 1], bf16, tag="v")
            nc.vector.memset(vt[:, :, DH:DH + 1], 1.0)
            v_t.append(vt)
        for g in range(2):
            wt = wpool.tile([128, 8, 512], bf16, tag="w")
            hw.dma_start(out=wt, in_=d["wqkv"][l, 4 + g])
            for n in range(4):
                ps = ps_gemm.tile([128, S], f32, tag="gemm")
                for c in range(NE):
                    last = (c == NE - 1) and not flags["bqkv"]
                    mm(ps, h_t[c][:, n * 128:(n + 1) * 128], wt[:, c, :],
                       start=(c == 0), stop=last)
                if flags["bqkv"]:
                    mm(ps, env["ones_row"][:, :128],
                       bv_row[:, g * 512:(g + 1) * 512],
                       start=False, stop=True)
                nc.scalar.activation(
                    v_t[n][:, g * 8:(g + 1) * 8, 0:DH],
                    ps.rearrange("p (a b) -> p a b", a=8), AF.Copy)

    if l == 0 and "dbg_v" in d:
        for n in range(4):
            hw.dma_start(out=d["dbg_v"][n], in_=v_t[n])

    # --- attention ------------------------------------------------------------
    with nc.named_scope(f"L{l}_attn"):
        wo_ts = []
        for wh in range(2):
            wt = wpool.tile([128, 4, 8, 128], bf16, tag="w", name=f"wo{l}_{wh}")
            hw.dma_start(out=wt, in_=d["wo"][l][:, wh * 4:(wh + 1) * 4])
            wo_ts.append(wt)
        ctxp = [ctxpool.tile([128, S], bf16, tag="ctx", name=f"cp{l}_{i}")
                for i in range(8)]
        at_q = {}

        def emit_scores(h):
            qt = qk_t[h // 2]
            kt = qk_t[8 + h // 2]
            r0 = (h % 2) * DH
            ath = atpool.tile([128, EM_W], bf16, tag="at", name=f"a{l}_{h}")
            for kc in range(4):
                qlo = kc * 128
                N = S - qlo
                sps = ps_gemm.tile([128, S], f32, tag="gemm", name=f"s{l}_{h}_{kc}")
                mm(sps[:, :N], kt[r0:r0 + DH, kc * 128:(kc + 1) * 128],
                   qt[r0:r0 + DH, qlo:S], start=True, stop=True)
                nc.scalar.activation(ath[:, EM_OFF[kc]:EM_OFF[kc] + N],
                                     sps[:, :N], AF.Exp)
            nc.vector.tensor_mul(ath, ath,
                                 em_sb[:, h * EM_W:(h + 1) * EM_W])
            at_q[h] = ath

        def emit_av(h):
            ath = at_q.pop(h)
            cps = ps_ctx.tile([DH + 1, S], f32, tag="ctxps", name=f"c{l}_{h}")
            for kc in range(4):
                qlo = kc * 128
                N = S - qlo
                mm(cps[:, qlo:S], v_t[kc][:, h, :],
                   ath[:, EM_OFF[kc]:EM_OFF[kc] + N],
                   start=(kc == 0), stop=(kc == 3), skip_group_check=True)
            srow = smallf.tile([1, S], f32, tag="sf", name=f"sr{l}_{h}")
            nc.scalar.activation(srow, cps[DH:DH + 1, :], AF.Copy)
            rec = smallf.tile([1, S], f32, tag="sf", name=f"re{l}_{h}")
            nc.vector.reciprocal_approx_fast(out=rec, in_=srow)
            recR = recpool.tile([DH, S], f32, tag="rec", name=f"rr{l}_{h}")
            nc.gpsimd.partition_broadcast(recR, rec, channels=DH)
            hp = h // 2
            if h % 2 == 0:
                nc.vector.tensor_mul(ctxp[hp][0:DH, :], cps[0:DH, :], recR)
            else:
                chh = tmppool.tile([DH, S], bf16, tag="ate", name=f"ch{l}_{h}")
                nc.vector.tensor_mul(chh, cps[0:DH, :], recR)
                nc.vector.tensor_copy(ctxp[hp][DH:128, :], chh)

        emit_scores(0)
        emit_scores(1)
        emit_scores(2)
        for h in range(3, H):
            emit_scores(h)
            emit_av(h - 3)
        scrap = smallf.tile([1, 1], f32, tag="scrap", bufs=2, name=f"scr{l}a")
        nc.scalar.activation(scrap, at_q[H - 1][:1, :1], AF.Abs_reciprocal_sqrt)
        emit_av(H - 3)
        emit_av(H - 2)
        emit_av(H - 1)

        if l == 0 and "dbg_at" in d:
            pass  # at tiles are popped; skip
        if l == 0 and "dbg_ctx" in d:
            for i in range(8):
                hw.dma_start(out=d["dbg_ctx"][i], in_=ctxp[i])

        # out-proj (paired heads, K=128) + residual
        r1_t = []
        lnacc1 = _LNAcc(nc, env, f"l{l}a")
        for wave in range(2):
            pss = [ps_gemm.tile([128, S], f32, tag="gemm",
                                name=f"wops{l}_{wave}_{i}") for i in range(4)]
            for hp in range(8):
                for i in range(4):
                    mm(pss[i], wo_ts[hp // 4][:, hp % 4, wave * 4 + i, :],
                       ctxp[hp], start=(hp == 0), stop=(hp == 7))
            for i in range(4):
                mi = wave * 4 + i
                r1 = hpool.tile([128, S], bf16, tag="h", name=f"r1_{l}_{mi}")
                bsc = bo_pp[:, mi:mi + 1] if flags["bo"] else 0.0
                nc.vector.scalar_tensor_tensor(r1, pss[i], bsc, h_t[mi],
                                               OP.add, OP.add)
                lnacc1.add(mi, r1)
                if l == 0 and "dbg_r1" in d:
                    hw.dma_start(out=d["dbg_r1"][mi], in_=r1)
                r1_t.append(r1)

    with nc.named_scope(f"L{l}_ln1"):
        rstdR, uR = lnacc1.stats()
        h1_t = _ln_normalize(nc, env, r1_t, rstdR, uR, ln_s[0], ln_b[0])
        if l == 0 and "dbg_h1" in d:
            for c in range(NE):
                hw.dma_start(out=d["dbg_h1"][c], in_=h1_t[c])

    # --- FFN -----------------------------------------------------------------
    with nc.named_scope(f"L{l}_ffn"):
        scrapg = smallf.tile([1, 1], f32, tag="scrap", bufs=2, name=f"scr{l}g")
        nc.scalar.activation(scrapg, h1_t[0][:1, :1], AF.Gelu)
        ff_t = []
        for g in range(8):
            wt = wpool.tile([128, 8, 512], bf16, tag="w")
            hw.dma_start(out=wt, in_=d["w1"][l, g])
            for mi in range(4):
                mt = g * 4 + mi
                ps = ps_gemm.tile([128, S], f32, tag="gemm")
                for c in range(NE):
                    mm(ps, wt[:, c, mi * 128:(mi + 1) * 128], h1_t[c],
                       start=(c == 0), stop=(c == NE - 1))
                ft = ffpool.tile([128, S], bf16, tag="ff")
                if flags["b1"]:
                    nc.scalar.activation(ft, ps, AF.Gelu,
                                         bias=b1_pp[:, mt:mt + 1])
                else:
                    nc.scalar.activation(ft, ps, AF.Gelu)
                ff_t.append(ft)
        scrap2 = smallf.tile([1, 1], f32, tag="scrap", bufs=2, name=f"scr{l}b")
        nc.scalar.activation(scrap2, ff_t[31][:1, :1], AF.Abs_reciprocal_sqrt)
        r2_t = [None] * NE
        lnacc2 = _LNAcc(nc, env, f"l{l}f")
        for half in range(2):
            pss = [ps_gemm.tile([128, S], f32, tag="gemm",
                                name=f"ff2ps{l}_{half}_{i}") for i in range(4)]
            for cg in range(4):
                wt = wpool.tile([128, 8, 512], bf16, tag="w")
                hw.dma_start(out=wt, in_=d["w2"][l, half, cg])
                for c8 in range(8):
                    c = cg * 8 + c8
                    for mi in range(4):
                        mm(pss[mi], wt[:, c8, mi * 128:(mi + 1) * 128], ff_t[c],
                           start=(c == 0), stop=(c == 31))
            for mi in range(4):
                mt = half * 4 + mi
                r2 = hpool.tile([128, S], bf16, tag="h")
                bsc = b2_pp[:, mt:mt + 1] if flags["b2"] else 0.0
                nc.vector.scalar_tensor_tensor(r2, pss[mi], bsc, h1_t[mt],
                                               OP.add, OP.add)
                lnacc2.add(mt, r2)
                r2_t[mt] = r2

    if l == L - 1:
        genw_sb = []
        for gi in range(4):
            wt = wpool.tile([128, 4, 640], bf16, tag="w", name=f"genw{gi}")
            hw.dma_start(out=wt, in_=d["genw"][gi])
            genw_sb.append(wt)
        env["genw_sb"] = genw_sb

    with nc.named_scope(f"L{l}_ln2"):
        rstdR, uR = lnacc2.stats()
        if l < L - 1:
            scrap3 = smallf.tile([1, 1], f32, tag="scrap", bufs=2,
                                 name=f"scr{l}c")
            nc.scalar.activation(scrap3, rstdR[:1, :1], AF.Exp)
        nxt = None
        if lnacc_f is not None:
            lnacc_f[0] = _LNAcc(nc, env, "f", pool=env["ps_gemm"], tag="gemm")
            nxt = lnacc_f[0]
        h2_t = _ln_normalize(nc, env, r2_t, rstdR, uR, ln_s[1], ln_b[1],
                             nxt=nxt)
        if l == 0 and "dbg_h2" in d:
            for c in range(NE):
                hw.dma_start(out=d["dbg_h2"][c], in_=h2_t[c])
    return h2_t


def _build(flags):
    key = tuple(sorted(flags.items()))
    if key in _CACHE:
        return _CACHE[key]
    from contextlib import ExitStack

    nc = bacc.Bacc("TRN2", debug=False)
    d = _declare(nc, flags)
    with tile.TileContext(nc) as tc:
        with ExitStack() as ctx:
            _emit(nc, tc, d, ctx, flags)
    nc.compile()
    _CACHE[key] = nc
    return nc


def kernel_internal(inputs, trace=False, trace_kwargs=None):
    shared, flags = _prep_shared(inputs)
    cores = _prep_percore(inputs)
    nc = _build(flags)
    in_maps = []
    for b in range(B):
        m = dict(shared)
        m.update(cores[b])
        in_maps.append(m)
    res = run_bass_kernel_spmd(
        nc, in_maps, core_ids=list(range(B)), trace=trace,
        **(trace_kwargs or {}),
    )
    outs = []
    for b in range(B):
        lo = res.results[b]["logits"]  # [10, 128, 512]
        lo = lo.reshape(NO * 128, S)[:VV * VR].T  # [512, 1200]
        outs.append(lo)
    out = np.stack(outs).astype(np.float32)  # [B, S, 1200]
    return out, res


def kernel(**inputs):
    out, _ = kernel_internal(inputs)
    return out


# revision 31
# speedup vs baseline: 1.0026x; 1.0026x over previous
"""Trainium2 Bass kernel for nn_BaseGenerator (4-layer dense transformer).

Strategy: pure data-parallel over batch (B=8 -> 8 NeuronCores, no
collectives).  Each core runs the full transformer on one batch element.
Activations are kept feature-major [E, S] in bf16 so every GEMM contracts
over the partition dim; PSUM accumulates in fp32.

Key performance structure:
  - exp(mask) shipped from host, resident in SBUF for all 4 layers;
    applied MULTIPLICATIVELY on the Vector engine (at = exp(s) * expmask),
    so no PE matmuls or per-layer DMA for masking.
  - causal truncation: score/AV matmuls + exp only cover q >= kc*128 for
    key chunk kc (62.5% of full work).
  - Wo contracts K=128 over paired heads (ctx stored as [128, S] pairs).
  - All plain PSUM->SBUF copies/bias/residual ops run on Vector (DVE);
    Scalar engine only runs Exp / Gelu / Sqrt, avoiding activation-table
    thrashing.
  - token embeddings (val+ring gather) precomputed on host.
  - zero biases / unit LN scales detected host-side and elided.
"""

import os
import sys

for _p in ("/opt/trn_rl_repo",):
    if _p not in sys.path:
        sys.path.insert(0, _p)

import ml_dtypes
import numpy as np

import concourse.bass as bass
import concourse.mybir as mybir
import concourse.tile as tile
from concourse import bacc
from concourse.bass_utils import run_bass_kernel_spmd

BF16 = ml_dtypes.bfloat16

L, E, H, F = 4, 1024, 16, 4096
B, S = 8, 512
VV, VR = 40, 30
DIST_V = 200
PAD_ID = 0
DH = E // H  # 64
NE = E // 128  # 8 feature chunks
NO = 10  # logit row tiles (1280 padded)
EM_OFF = (0, 512, 896, 1152)  # causal-packed expmask offsets per kc
EM_W = 1280
FULLW = False

f32 = mybir.dt.float32
bf16 = mybir.dt.bfloat16
AF = mybir.ActivationFunctionType
OP = mybir.AluOpType

_CACHE = {}


# ----------------------------------------------------------------------------
# host-side input prep
# ----------------------------------------------------------------------------

def _b16(x):
    return np.ascontiguousarray(np.asarray(x).astype(BF16))


def _block_lhsT(W, gsize):
    # W: [L?, OUT, IN] -> [.., G, 128, IN//128, gsize] with
    # out[..., g, p, c, o] = W[..., g*gsize + o, c*128 + p]
    *lead, O, I = W.shape
    G = O // gsize
    nc_ = I // 128
    Wb = W.reshape(*lead, G, gsize, nc_, 128)
    Wb = np.moveaxis(Wb, -1, -3)  # [..., G, 128, gsize, nc]
    Wb = np.swapaxes(Wb, -1, -2)  # [..., G, 128, nc, gsize]
    return np.ascontiguousarray(Wb)


def _pp(v):  # [..., N*128] -> [..., 128, N]
    *lead, N = v.shape
    return np.ascontiguousarray(
        v.reshape(*lead, N // 128, 128).swapaxes(-1, -2).astype(np.float32)
    )


def _prep_shared(inp):
    """Weight-layout prep shared by all cores. Returns (tensors, flags)."""
    out = {}
    flags = {}

    Wqkv = np.asarray(inp["Wqkv"], np.float32).copy()  # [L, 3E, E]
    bqkv = np.asarray(inp["bqkv"], np.float32).copy()  # [L, 3E]
    # fold attention scale into Q projection
    scale = 1.0 / np.sqrt(DH)
    Wqkv[:, :E, :] *= scale
    bqkv[:, :E] *= scale

    out["wqkv"] = _b16(_block_lhsT(Wqkv, 512))          # [L, 6, 128, 8, 512]
    # negated input-dim sums of the (scaled) Q,K projections, for the
    # LN-deferred QKV fixup: qk = ps*rstd - wsum*(u*rstd)
    out["wqksum_pp"] = _pp(-Wqkv[:, :2 * E, :].sum(-1))  # [L, 128, 16]

    # Wo paired-head lhsT: wo[l, p, hp, mi, m] = Wo[l, mi*128+m, hp*128+p]
    Wo = np.asarray(inp["Wo"], np.float32)  # [L, E(out), E(in=ctx)]
    wo = Wo.reshape(L, 8, 128, 8, 128)      # [l, mi, m, hp, p]
    wo = wo.transpose(0, 4, 3, 1, 2)        # [l, p, hp, mi, m]
    out["wo"] = _b16(wo)                    # [L, 128, 8, 8, 128]

    out["w1"] = _b16(_block_lhsT(np.asarray(inp["W1"], np.float32), 512))
    W2 = np.asarray(inp["W2"], np.float32)  # out=E, in=F
    w2b = _block_lhsT(W2, 512)  # [L, 2, 128, 32, 512]
    w2b = w2b.reshape(L, 2, 128, 4, 8, 512).transpose(0, 1, 3, 2, 4, 5)
    out["w2"] = _b16(w2b)  # [L, 2, 4, 128, 8, 512]

    genW = np.asarray(inp["gen_W"], np.float32)  # [1200, E]
    genW_pad = np.zeros((1280, E), np.float32)
    genW_pad[:1200] = genW
    genw_b = _block_lhsT(genW_pad, 640).reshape(2, 128, 2, 4, 640)
    genw_b = genw_b.transpose(0, 2, 1, 3, 4).reshape(4, 128, 4, 640)
    out["genw"] = _b16(genw_b)  # [4, 128, 4, 640]

    # biases (elided when zero)
    flags["bqkv"] = bool(np.any(bqkv != 0))
    flags["bo"] = bool(np.any(np.asarray(inp["bo"]) != 0))
    flags["b1"] = bool(np.any(np.asarray(inp["b1"]) != 0))
    flags["b2"] = bool(np.any(np.asarray(inp["b2"]) != 0))
    flags["genb"] = bool(np.any(np.asarray(inp["gen_b"]) != 0))
    flags["ln_s"] = not bool(
        np.all(np.asarray(inp["ln1_s"]) == 1) and np.all(np.asarray(inp["ln2_s"]) == 1)
    )
    flags["ln_b"] = bool(
        np.any(np.asarray(inp["ln1_b"]) != 0) or np.any(np.asarray(inp["ln2_b"]) != 0)
    )
    flags["lnf_s"] = not bool(np.all(np.asarray(inp["lnf_s"]) == 1))
    flags["lnf_b"] = bool(np.any(np.asarray(inp["lnf_b"]) != 0))

    if flags["bqkv"]:
        out["bqkv_pp"] = _pp(bqkv[:, : 2 * E])  # [L, 128, 16] (Q scaled)
        out["bv_row"] = _b16(bqkv[:, 2 * E:].reshape(L, 1, E))  # [L, 1, E]
        out["ones_row"] = _b16(np.ones((1, S), np.float32))
    if flags["bo"]:
        out["bo_pp"] = _pp(np.asarray(inp["bo"], np.float32))  # [L, 128, 8]
    if flags["b1"]:
        out["b1_pp"] = _pp(np.asarray(inp["b1"], np.float32))  # [L, 128, 32]
    if flags["b2"]:
        out["b2_pp"] = _pp(np.asarray(inp["b2"], np.float32))  # [L, 128, 8]
    if flags["genb"]:
        gbp = np.zeros((1280,), np.float32)
        gbp[:1200] = np.asarray(inp["gen_b"], np.float32)
        out["gen_b_pp"] = np.ascontiguousarray(gbp.reshape(NO, 128).T)  # [128, 10]
    if flags["ln_s"]:
        ln_s = np.stack([np.asarray(inp["ln1_s"], np.float32),
                         np.asarray(inp["ln2_s"], np.float32)], 1)  # [L, 2, E]
        out["ln_s_pp"] = _pp(ln_s)  # [L, 2, 128, 8]
    if flags["ln_b"]:
        ln_b = np.stack([np.asarray(inp["ln1_b"], np.float32),
                         np.asarray(inp["ln2_b"], np.float32)], 1)
        out["ln_b_pp"] = _pp(ln_b)
    if flags["lnf_s"]:
        out["lnf_s_pp"] = _pp(np.asarray(inp["lnf_s"], np.float32))  # [128, 8]
    if flags["lnf_b"]:
        out["lnf_b_pp"] = _pp(np.asarray(inp["lnf_b"], np.float32))

    out["ones_col"] = _b16(np.ones((128, 1), np.float32))
    out["ones_r128"] = _b16(np.ones((1, 128), np.float32))
    return out, flags


def _prep_percore(inp):
    """Per-core tensors: embedded tokens + packed exp(mask)."""
    val = np.asarray(inp["val_sequences"]).astype(np.int64)    # [B, S]
    ring = np.asarray(inp["ring_sequences"]).astype(np.int64)  # [B, S]
    dist = np.asarray(inp["distance_squares"]).astype(np.int64)  # [B, S, S]
    de = np.asarray(inp["dist_emb"], np.float32)  # [200, H]
    ve = np.asarray(inp["val_emb"], np.float32)   # [VV, E]
    re = np.asarray(inp["ring_emb"], np.float32)  # [VR, E]

    # x0: feature-major embedded tokens [B, NE, 128, S]
    x0 = (ve[val] + re[ring]) * np.sqrt(E)                  # [B, S, E]
    x0 = x0.reshape(B, S, NE, 128).transpose(0, 2, 3, 1)    # [B, NE, 128, S]
    x0 = _b16(x0)

    # expmask, causal-packed: em[b, p, h*1280 + OFF[kc] + j]
    #   = exp(de[dist[b, qlo+j, kc*128+p]])  (0 where k>q or key is pad)
    g = np.exp(de[dist])                 # [B, q, k, H]
    kk = np.arange(S)
    keep = (kk[None, :] <= kk[:, None])  # [q, k]: k <= q
    g *= keep[None, :, :, None]
    g *= (val != PAD_ID)[:, None, :, None]
    emp = np.zeros((B, H, 128, EM_W), np.float32)
    for kc in range(4):
        qlo = kc * 128
        w = S - qlo
        blk = g[:, qlo:, kc * 128:(kc + 1) * 128, :]  # [B, w, 128, H]
        emp[:, :, :, EM_OFF[kc]:EM_OFF[kc] + w] = blk.transpose(0, 3, 2, 1)
    emp = _b16(emp.transpose(0, 2, 1, 3).reshape(B, 128, H * EM_W))

    cores = []
    for b in range(B):
        cores.append({"x0": np.ascontiguousarray(x0[b]),
                      "emask": np.ascontiguousarray(emp[b])})
    return cores


# ----------------------------------------------------------------------------
# device program
# ----------------------------------------------------------------------------

def _declare(nc, flags):
    d = {}

    def di(name, shape, dt):
        d[name] = nc.dram_tensor(name, list(shape), dt, kind="ExternalInput").ap()

    di("wqkv", (L, 6, 128, 8, 512), bf16)
    di("wqksum_pp", (L, 128, 16), f32)
    di("wo", (L, 128, 8, 8, 128), bf16)
    di("w1", (L, 8, 128, 8, 512), bf16)
    di("w2", (L, 2, 4, 128, 8, 512), bf16)
    di("genw", (4, 128, 4, 640), bf16)
    if flags["bqkv"]:
        di("bqkv_pp", (L, 128, 16), f32)
        di("bv_row", (L, 1, E), bf16)
        di("ones_row", (1, S), bf16)
    if flags["bo"]:
        di("bo_pp", (L, 128, 8), f32)
    if flags["b1"]:
        di("b1_pp", (L, 128, 32), f32)
    if flags["b2"]:
        di("b2_pp", (L, 128, 8), f32)
    if flags["genb"]:
        di("gen_b_pp", (128, NO), f32)
    if flags["ln_s"]:
        di("ln_s_pp", (L, 2, 128, 8), f32)
    if flags["ln_b"]:
        di("ln_b_pp", (L, 2, 128, 8), f32)
    if flags["lnf_s"]:
        di("lnf_s_pp", (128, 8), f32)
    if flags["lnf_b"]:
        di("lnf_b_pp", (128, 8), f32)
    di("ones_col", (128, 1), bf16)
    di("ones_r128", (1, 128), bf16)
    di("x0", (NE, 128, S), bf16)
    di("emask", (128, H * EM_W), bf16)
    d["logits"] = nc.dram_tensor(
        "logits", [NO, 128, S], f32, kind="ExternalOutput"
    ).ap()
    if os.environ.get("BG_DEBUG"):
        def do(name, shape):
            d[name] = nc.dram_tensor(name, list(shape), bf16,
                                     kind="ExternalOutput").ap()
        do("dbg_h0", (NE, 128, S))
        do("dbg_qk", (16, 128, S))
        do("dbg_v", (4, 128, H, DH + 1))
        do("dbg_at", (8, 128, S))
        do("dbg_ctx", (8, 128, S))
        do("dbg_r1", (NE, 128, S))
        do("dbg_h1", (NE, 128, S))
        do("dbg_h2", (NE, 128, S))
    return d


def _emit(nc, tc, d, ctx, flags):
    mm = nc.tensor.matmul

    cpool = ctx.enter_context(tc.tile_pool(name="cpool", bufs=1))
    wpool = ctx.enter_context(tc.tile_pool(name="wpool", bufs=4))
    hpool = ctx.enter_context(tc.tile_pool(name="hpool", bufs=16))
    qkpool = ctx.enter_context(tc.tile_pool(name="qkpool", bufs=17))
    vpool = ctx.enter_context(tc.tile_pool(name="vpool", bufs=4))
    atpool = ctx.enter_context(tc.tile_pool(name="atpool", bufs=5))
    ctxpool = ctx.enter_context(tc.tile_pool(name="ctxpool", bufs=8))
    ffpool = ctx.enter_context(tc.tile_pool(name="ffpool", bufs=32))
    tmppool = ctx.enter_context(tc.tile_pool(name="tmppool", bufs=4))
    smallf = ctx.enter_context(tc.tile_pool(name="smallf", bufs=4))
    smallb = ctx.enter_context(tc.tile_pool(name="smallb", bufs=2))
    recpool = ctx.enter_context(tc.tile_pool(name="recpool", bufs=2))
    outpool = ctx.enter_context(tc.tile_pool(name="outpool", bufs=2))
    pppool = ctx.enter_context(tc.tile_pool(name="pppool", bufs=4))
    empool = ctx.enter_context(tc.tile_pool(name="empool", bufs=1))

    ps_gemm = ctx.enter_context(tc.tile_pool(name="ps_gemm", bufs=4, space="PSUM"))
    ps_ctx = ctx.enter_context(tc.tile_pool(name="ps_ctx", bufs=3, space="PSUM"))
    ps_ln = ctx.enter_context(tc.tile_pool(name="ps_ln", bufs=1, space="PSUM"))

    hw = nc.sync  # HWDGE dma engine

    # --- constants -----------------------------------------------------------
    ones_col = cpool.tile([128, 1], bf16)
    hw.dma_start(out=ones_col, in_=d["ones_col"])
    ones_r128 = cpool.tile([1, 128], bf16)
    hw.dma_start(out=ones_r128, in_=d["ones_r128"])
    eps_t = cpool.tile([128, 1], f32)
    nc.vector.memset(eps_t, 1e-5)
    genb_pp = None
    if flags["genb"]:
        genb_pp = cpool.tile([128, NO], f32)
        hw.dma_start(out=genb_pp, in_=d["gen_b_pp"])
    lnf_s = lnf_b = None
    if flags["lnf_s"]:
        lnf_s = cpool.tile([128, 8], f32)
        hw.dma_start(out=lnf_s, in_=d["lnf_s_pp"])
    if flags["lnf_b"]:
        lnf_b = cpool.tile([128, 8], f32)
        hw.dma_start(out=lnf_b, in_=d["lnf_b_pp"])
    ones_row = bv_all = None
    if flags["bqkv"]:
        ones_row = cpool.tile([1, S], bf16)
        hw.dma_start(out=ones_row, in_=d["ones_row"])

    # resident expmask (on SWDGE queue so it doesn't delay weight loads)
    em_sb = empool.tile([128, H * EM_W], bf16)
    nc.gpsimd.dma_start(out=em_sb, in_=d["emask"])

    # --- embedding (host-precomputed) ---------------------------------------
    h_t = []
    for c in range(NE):
        ht = hpool.tile([128, S], bf16, tag="h")
        hw.dma_start(out=ht, in_=d["x0"][c])
        if "dbg_h0" in d:
            nc.sync.dma_start(out=d["dbg_h0"][c], in_=ht)
        h_t.append(ht)

    env = dict(locals())
    env["ps_ln"] = ps_ln

    # --- layers --------------------------------------------------------------
    # When ln2 has unit scale / zero bias, its output is exactly normalized
    # (mean 0, var 1-eps'), so a unit-scale/zero-bias final LN is an identity
    # up to O(eps) -- skip it.
    skip_lnf = not (flags["lnf_s"] or flags["lnf_b"]
                    or flags["ln_s"] or flags["ln_b"])
    lnacc_f = [None]
    qkv_src = None
    for l in range(L):
        h_t, qkv_src = _layer(
            nc, tc, d, l, h_t, env, flags, qkv_src=qkv_src,
            lnacc_f=lnacc_f if (l == L - 1 and not skip_lnf) else None)

    # --- final LN + head -----------------------------------------------------
    with nc.named_scope("final"):
        if skip_lnf:
            hf = h_t
        else:
            rstdR, uR = lnacc_f[0].stats()
            hf = _ln_normalize(nc, env, h_t, rstdR, uR, lnf_s, lnf_b)
        genw_sb = env["genw_sb"]
        for mt in range(NO):
            g, mi = divmod(mt, 5)
            ps = ps_gemm.tile([128, S], f32, tag="gemm")
            for c in range(NE):
                mm(ps, genw_sb[g * 2 + c // 4][:, c % 4, mi * 128:(mi + 1) * 128],
                   hf[c],
                   start=(c == 0), stop=(c == NE - 1))
            ot = outpool.tile([128, S], f32, tag="f32out")
            if flags["genb"]:
                nc.vector.tensor_scalar(ot, ps, genb_pp[:, mt:mt + 1], None, OP.add)
            else:
                nc.vector.tensor_copy(ot, ps)
            hw.dma_start(out=d["logits"][mt], in_=ot)


class _LNAcc:
    """Pipelined LN statistics: sum/sumsq matmuls emitted as chunks land."""

    def __init__(self, nc, env, name, pool=None, tag="ln"):
        self.nc = nc
        self.env = env
        self.name = name
        pool = pool if pool is not None else env["ps_ln"]
        sums = pool.tile([33, S], f32, tag=tag, name=f"lns_{name}")
        self.sums_r = sums[0:1, :]
        self.sums_q = sums[32:33, :]

    def add(self, c, r_tile):
        nc, env = self.nc, self.env
        sq = env["tmppool"].tile([128, S], bf16, tag="sq",
                                 name=f"sq_{self.name}_{c}")
        nc.scalar.activation(sq, r_tile, AF.Square)
        mm = nc.tensor.matmul
        mm(self.sums_r, env["ones_col"], r_tile, start=(c == 0),
           stop=(c == NE - 1))
        mm(self.sums_q, env["ones_col"], sq, start=(c == 0),
           stop=(c == NE - 1))

    def stats(self, sbuf_bcast=False):
        nc, env = self.nc, self.env
        smallf = env["smallf"]; smallb = env["smallb"]; recpool = env["recpool"]
        nm = self.name
        s2 = smallf.tile([1, S], f32, tag="sf", name=f"s2_{nm}")
        nc.scalar.activation(s2, self.sums_r, AF.Square)
        varE = smallf.tile([1, S], f32, tag="sf", name=f"ve_{nm}")
        # varE = sumsq - s2/E  (= E * var)
        nc.vector.scalar_tensor_tensor(varE, s2, -1.0 / E, self.sums_q,
                                       OP.mult, OP.add)
        rstd = smallf.tile([1, S], f32, tag="sf", name=f"rs_{nm}")
        nc.scalar.activation(rstd, varE, AF.Abs_reciprocal_sqrt,
                             bias=env["eps_t"][:1, :], scale=1.0 / E)
        ru = smallb.tile([1, 2 * S], bf16, tag="sb", name=f"ru_{nm}")
        nc.vector.tensor_copy(ru[:, 0:S], rstd)
        # u*rstd = (sum/E) * rstd
        nc.vector.scalar_tensor_tensor(ru[:, S:2 * S], self.sums_r, 1.0 / E,
                                       rstd, OP.mult, OP.mult)
        if sbuf_bcast:
            ruR = recpool.tile([128, 2 * S], bf16, tag="rec", name=f"ruR_{nm}")
            nc.gpsimd.partition_broadcast(ruR, ru, channels=128)
            return ruR[:, 0:S], ruR[:, S:2 * S]
        mm = nc.tensor.matmul
        ps_ctx = env["ps_ctx"]
        rstdR = ps_ctx.tile([128, S], f32, tag="ctxps", name=f"bcr_{nm}")
        mm(rstdR, env["ones_r128"], ru[:, 0:S], start=True, stop=True)
        uR = ps_ctx.tile([128, S], f32, tag="ctxps", name=f"bcu_{nm}")
        mm(uR, env["ones_r128"], ru[:, S:2 * S], start=True, stop=True)
        return rstdR, uR


def _ln_normalize(nc, env, r_t, rstdR, uR, s_pp, b_pp, nxt=None):
    out_t = []
    for c in range(NE):
        t2 = env["tmppool"].tile([128, S], bf16, tag="tmp")
        nc.vector.tensor_sub(t2, r_t[c], uR)
        ht = env["hpool"].tile([128, S], bf16, tag="h")
        sc = s_pp[:, c:c + 1] if s_pp is not None else 1.0
        nc.vector.scalar_tensor_tensor(ht, t2, sc, rstdR, OP.mult, OP.mult)
        if b_pp is not None:
            nc.vector.tensor_scalar(ht, ht, b_pp[:, c:c + 1], None, OP.add)
        if nxt is not None:
            nxt.add(c, ht)
        out_t.append(ht)
    return out_t


def _layer(nc, tc, d, l, h_t, env, flags, qkv_src=None, lnacc_f=None):
    mm = nc.tensor.matmul
    hw = nc.sync
    wpool = env["wpool"]; hpool = env["hpool"]; qkpool = env["qkpool"]
    vpool = env["vpool"]; atpool = env["atpool"]
    ctxpool = env["ctxpool"]; ffpool = env["ffpool"]; tmppool = env["tmppool"]
    smallf = env["smallf"]; smallb = env["smallb"]; recpool = env["recpool"]
    pppool = env["pppool"]
    ps_gemm = env["ps_gemm"]; ps_ctx = env["ps_ctx"]
    ones_col = env["ones_col"]; em_sb = env["em_sb"]

    # per-layer small params
    bqkv_pp = bv_row = bo_pp = b1_pp = b2_pp = None
    ln_s = [None, None]
    ln_b = [None, None]
    if flags["bqkv"]:
        bqkv_pp = pppool.tile([128, 16], f32, tag="pp16")
        hw.dma_start(out=bqkv_pp, in_=d["bqkv_pp"][l])
        bv_row = pppool.tile([1, E], bf16, tag="bvrow", bufs=2)
        hw.dma_start(out=bv_row, in_=d["bv_row"][l])
    if flags["bo"]:
        bo_pp = pppool.tile([128, 8], f32, tag="pp8")
        hw.dma_start(out=bo_pp, in_=d["bo_pp"][l])
    if flags["b1"]:
        b1_pp = pppool.tile([128, 32], f32, tag="pp32")
        hw.dma_start(out=b1_pp, in_=d["b1_pp"][l])
    if flags["b2"]:
        b2_pp = pppool.tile([128, 8], f32, tag="pp8")
        hw.dma_start(out=b2_pp, in_=d["b2_pp"][l])
    if flags["ln_s"]:
        ln_s = [pppool.tile([128, 8], f32, tag="pp8", name=f"lns{l}_{i}")
                for i in range(2)]
        for i in range(2):
            hw.dma_start(out=ln_s[i], in_=d["ln_s_pp"][l, i])
    if flags["ln_b"]:
        ln_b = [pppool.tile([128, 8], f32, tag="pp8", name=f"lnb{l}_{i}")
                for i in range(2)]
        for i in range(2):
            hw.dma_start(out=ln_b[i], in_=d["ln_b_pp"][l, i])

    # --- QKV -----------------------------------------------------------------
    with nc.named_scope(f"L{l}_qkv"):
        qk_t = []  # 16 tiles: q 0..7 (2 heads each), k 8..15
        if qkv_src is not None:
            qkv_rhs, fx_rstd, fx_urstd, fx_negw = qkv_src
        else:
            qkv_rhs = h_t
        for g in range(4):  # Q, K feature-major
            wt = wpool.tile([128, 8, 512], bf16, tag="w")
            hw.dma_start(out=wt, in_=d["wqkv"][l, g])
            for mi in range(4):
                mt = g * 4 + mi
                ps = ps_gemm.tile([128, S], f32, tag="gemm")
                for c in range(NE):
                    mm(ps, wt[:, c, mi * 128:(mi + 1) * 128], qkv_rhs[c],
                       start=(c == 0), stop=(c == NE - 1))
                qk = qkpool.tile([128, S], bf16, tag="qk")
                if qkv_src is not None:
                    # LN-deferred fixup: qk = ps*rstd - wsum*(u*rstd)
                    fxt = tmppool.tile([128, S], bf16, tag="tmp",
                                       name=f"fx{l}_{mt}")
                    nc.vector.tensor_mul(fxt, ps, fx_rstd)
                    nc.vector.scalar_tensor_tensor(
                        qk, fx_urstd, fx_negw[:, mt:mt + 1], fxt,
                        OP.mult, OP.add)
                    if flags["bqkv"]:
                        nc.vector.tensor_scalar(qk, qk, bqkv_pp[:, mt:mt + 1],
                                                None, OP.add)
                elif flags["bqkv"]:
                    nc.scalar.activation(qk, ps, AF.Identity,
                                         bias=bqkv_pp[:, mt:mt + 1])
                else:
                    nc.scalar.activation(qk, ps, AF.Copy)
                if l == 0 and "dbg_qk" in d:
                    hw.dma_start(out=d["dbg_qk"][mt], in_=qk)
                qk_t.append(qk)
        # V token-major, augmented with ones column
        v_t = []
        for n in range(4):
            vt = vpool.tile([128, H, DH + 1], bf16, tag="v")
            nc.vector.memset(vt[:, :, DH:DH + 1], 1.0)
            v_t.append(vt)
        for g in range(2):
            wt = wpool.tile([128, 8, 512], bf16, tag="w")
            hw.dma_start(out=wt, in_=d["wqkv"][l, 4 + g])
            for n in range(4):
                ps = ps_gemm.tile([128, S], f32, tag="gemm")
                for c in range(NE):
                    last = (c == NE - 1) and not flags["bqkv"]
                    mm(ps, h_t[c][:, n * 128:(n + 1) * 128], wt[:, c, :],
                       start=(c == 0), stop=last)
                if flags["bqkv"]:
                    mm(ps, env["ones_row"][:, :128],
                       bv_row[:, g * 512:(g + 1) * 512],
                       start=False, stop=True)
                nc.scalar.activation(
                    v_t[n][:, g * 8:(g + 1) * 8, 0:DH],
                    ps.rearrange("p (a b) -> p a b", a=8), AF.Copy)

    if l == 0 and "dbg_v" in d:
        for n in range(4):
            hw.dma_start(out=d["dbg_v"][n], in_=v_t[n])

    # --- attention ------------------------------------------------------------
    with nc.named_scope(f"L{l}_attn"):
        wo_ts = []
        for wh in range(2):
            wt = wpool.tile([128, 4, 8, 128], bf16, tag="w", name=f"wo{l}_{wh}")
            hw.dma_start(out=wt, in_=d["wo"][l][:, wh * 4:(wh + 1) * 4])
            wo_ts.append(wt)
        ctxp = [ctxpool.tile([128, S], bf16, tag="ctx", name=f"cp{l}_{i}")
                for i in range(8)]
        at_q = {}

        def emit_scores(h):
            qt = qk_t[h // 2]
            kt = qk_t[8 + h // 2]
            r0 = (h % 2) * DH
            ath = atpool.tile([128, EM_W], bf16, tag="at", name=f"a{l}_{h}")
            for kc in range(4):
                qlo = kc * 128
                N = S - qlo
                sps = ps_gemm.tile([128, S], f32, tag="gemm", name=f"s{l}_{h}_{kc}")
                mm(sps[:, :N], kt[r0:r0 + DH, kc * 128:(kc + 1) * 128],
                   qt[r0:r0 + DH, qlo:S], start=True, stop=True)
                nc.scalar.activation(ath[:, EM_OFF[kc]:EM_OFF[kc] + N],
                                     sps[:, :N], AF.Exp)
            nc.vector.tensor_mul(ath, ath,
                                 em_sb[:, h * EM_W:(h + 1) * EM_W])
            at_q[h] = ath

        def emit_av(h):
            ath = at_q.pop(h)
            cps = ps_ctx.tile([DH + 1, S], f32, tag="ctxps", name=f"c{l}_{h}")
            for kc in range(4):
                qlo = kc * 128
                N = S - qlo
                mm(cps[:, qlo:S], v_t[kc][:, h, :],
                   ath[:, EM_OFF[kc]:EM_OFF[kc] + N],
                   start=(kc == 0), stop=(kc == 3), skip_group_check=True)
            srow = smallf.tile([1, S], f32, tag="sf", name=f"sr{l}_{h}")
            nc.scalar.activation(srow, cps[DH:DH + 1, :], AF.Copy)
            rec = smallf.tile([1, S], f32, tag="sf", name=f"re{l}_{h}")
            nc.vector.reciprocal_approx_fast(out=rec, in_=srow)
            recR = recpool.tile([DH, S], f32, tag="rec", name=f"rr{l}_{h}")
            nc.gpsimd.partition_broadcast(recR, rec, channels=DH)
            hp = h // 2
            if h % 2 == 0:
                nc.vector.tensor_mul(ctxp[hp][0:DH, :], cps[0:DH, :], recR)
            else:
                chh = tmppool.tile([DH, S], bf16, tag="ate", name=f"ch{l}_{h}")
                nc.vector.tensor_mul(chh, cps[0:DH, :], recR)
                nc.vector.tensor_copy(ctxp[hp][DH:128, :], chh)

        emit_scores(0)
        emit_scores(1)
        emit_scores(2)
        for h in range(3, H):
            emit_scores(h)
            emit_av(h - 3)
        scrap = smallf.tile([1, 1], f32, tag="scrap", bufs=2, name=f"scr{l}a")
        nc.scalar.activation(scrap, at_q[H - 1][:1, :1], AF.Abs_reciprocal_sqrt)
        emit_av(H - 3)
        emit_av(H - 2)
        emit_av(H - 1)

        if l == 0 and "dbg_at" in d:
            pass  # at tiles are popped; skip
        if l == 0 and "dbg_ctx" in d:
            for i in range(8):
                hw.dma_start(out=d["dbg_ctx"][i], in_=ctxp[i])

        # out-proj (paired heads, K=128) + residual
        r1_t = []
        lnacc1 = _LNAcc(nc, env, f"l{l}a")
        for wave in range(2):
            pss = [ps_gemm.tile([128, S], f32, tag="gemm",
                                name=f"wops{l}_{wave}_{i}") for i in range(4)]
            for hp in range(8):
                for i in range(4):
                    mm(pss[i], wo_ts[hp // 4][:, hp % 4, wave * 4 + i, :],
                       ctxp[hp], start=(hp == 0), stop=(hp == 7))
            for i in range(4):
                mi = wave * 4 + i
                r1 = hpool.tile([128, S], bf16, tag="h", name=f"r1_{l}_{mi}")
                bsc = bo_pp[:, mi:mi + 1] if flags["bo"] else 0.0
                nc.vector.scalar_tensor_tensor(r1, pss[i], bsc, h_t[mi],
                                               OP.add, OP.add)
                lnacc1.add(mi, r1)
                if l == 0 and "dbg_r1" in d:
                    hw.dma_start(out=d["dbg_r1"][mi], in_=r1)
                r1_t.append(r1)

    with nc.named_scope(f"L{l}_ln1"):
        rstdR, uR = lnacc1.stats()
        h1_t = _ln_normalize(nc, env, r1_t, rstdR, uR, ln_s[0], ln_b[0])
        if l == 0 and "dbg_h1" in d:
            for c in range(NE):
                hw.dma_start(out=d["dbg_h1"][c], in_=h1_t[c])

    # --- FFN -----------------------------------------------------------------
    with nc.named_scope(f"L{l}_ffn"):
        scrapg = smallf.tile([1, 1], f32, tag="scrap", bufs=2, name=f"scr{l}g")
        nc.scalar.activation(scrapg, h1_t[0][:1, :1], AF.Gelu)
        ff_t = []
        for g in range(8):
            wt = wpool.tile([128, 8, 512], bf16, tag="w")
            hw.dma_start(out=wt, in_=d["w1"][l, g])
            for mi in range(4):
                mt = g * 4 + mi
                ps = ps_gemm.tile([128, S], f32, tag="gemm")
                for c in range(NE):
                    mm(ps, wt[:, c, mi * 128:(mi + 1) * 128], h1_t[c],
                       start=(c == 0), stop=(c == NE - 1))
                ft = ffpool.tile([128, S], bf16, tag="ff")
                if flags["b1"]:
                    nc.scalar.activation(ft, ps, AF.Gelu,
                                         bias=b1_pp[:, mt:mt + 1])
                else:
                    nc.scalar.activation(ft, ps, AF.Gelu)
                ff_t.append(ft)
        scrap2 = smallf.tile([1, 1], f32, tag="scrap", bufs=2, name=f"scr{l}b")
        nc.scalar.activation(scrap2, ff_t[31][:1, :1], AF.Abs_reciprocal_sqrt)
        r2_t = [None] * NE
        lnacc2 = _LNAcc(nc, env, f"l{l}f")
        for half in range(2):
            pss = [ps_gemm.tile([128, S], f32, tag="gemm",
                                name=f"ff2ps{l}_{half}_{i}") for i in range(4)]
            for cg in range(4):
                wt = wpool.tile([128, 8, 512], bf16, tag="w")
                hw.dma_start(out=wt, in_=d["w2"][l, half, cg])
                for c8 in range(8):
                    c = cg * 8 + c8
                    for mi in range(4):
                        mm(pss[mi], wt[:, c8, mi * 128:(mi + 1) * 128], ff_t[c],
                           start=(c == 0), stop=(c == 31))
            for mi in range(4):
                mt = half * 4 + mi
                r2 = hpool.tile([128, S], bf16, tag="h")
                bsc = b2_pp[:, mt:mt + 1] if flags["b2"] else 0.0
                nc.vector.scalar_tensor_tensor(r2, pss[mi], bsc, h1_t[mt],
                                               OP.add, OP.add)
                lnacc2.add(mt, r2)
                r2_t[mt] = r2

    if l == L - 1:
        genw_sb = []
        for gi in range(4):
            wt = wpool.tile([128, 4, 640], bf16, tag="w", name=f"genw{gi}")
            hw.dma_start(out=wt, in_=d["genw"][gi])
            genw_sb.append(wt)
        env["genw_sb"] = genw_sb

    with nc.named_scope(f"L{l}_ln2"):
        defer = (l < L - 1) and not (flags["ln_s"] or flags["ln_b"])
        rstdR, uR = lnacc2.stats(sbuf_bcast=defer)
        if l < L - 1:
            scrap3 = smallf.tile([1, 1], f32, tag="scrap", bufs=2,
                                 name=f"scr{l}c")
            nc.scalar.activation(scrap3, rstdR[:1, :1], AF.Exp)
        nxt = None
        if lnacc_f is not None:
            lnacc_f[0] = _LNAcc(nc, env, "f", pool=env["ps_gemm"], tag="gemm")
            nxt = lnacc_f[0]
        h2_t = _ln_normalize(nc, env, r2_t, rstdR, uR, ln_s[1], ln_b[1],
                             nxt=nxt)
        if l == 0 and "dbg_h2" in d:
            for c in range(NE):
                hw.dma_start(out=d["dbg_h2"][c], in_=h2_t[c])
        qkv_src_next = None
        if defer:
            negw = pppool.tile([128, 16], f32, tag="pp16b", bufs=2,
                               name=f"negw{l + 1}")
            hw.dma_start(out=negw, in_=d["wqksum_pp"][l + 1])
            qkv_src_next = (r2_t, rstdR, uR, negw)
    return h2_t, qkv_src_next


def _build(flags):
    key = tuple(sorted(flags.items()))
    if key in _CACHE:
        return _CACHE[key]
    from contextlib import ExitStack

    nc = bacc.Bacc("TRN2", debug=False)
    d = _declare(nc, flags)
    with tile.TileContext(nc) as tc:
        with ExitStack() as ctx:
            _emit(nc, tc, d, ctx, flags)
    nc.compile()
    _CACHE[key] = nc
    return nc


def kernel_internal(inputs, trace=False, trace_kwargs=None):
    shared, flags = _prep_shared(inputs)
    cores = _prep_percore(inputs)
    nc = _build(flags)
    in_maps = []
    for b in range(B):
        m = dict(shared)
        m.update(cores[b])
        in_maps.append(m)
    res = run_bass_kernel_spmd(
        nc, in_maps, core_ids=list(range(B)), trace=trace,
        **(trace_kwargs or {}),
    )
    outs = []
    for b in range(B):
        lo = res.results[b]["logits"]  # [10, 128, 512]
        lo = lo.reshape(NO * 128, S)[:VV * VR].T  # [512, 1200]
        outs.append(lo)
    out = np.stack(outs).astype(np.float32)  # [B, S, 1200]
    return out, res


def kernel(**inputs):
    out, _ = kernel_internal(inputs)
    return out


# revision 33
# speedup vs baseline: 1.0122x; 1.0096x over previous
"""Trainium2 Bass kernel for nn_BaseGenerator (4-layer dense transformer).

Strategy: pure data-parallel over batch (B=8 -> 8 NeuronCores, no
collectives).  Each core runs the full transformer on one batch element.
Activations are kept feature-major [E, S] in bf16 so every GEMM contracts
over the partition dim; PSUM accumulates in fp32.

Key performance structure:
  - exp(mask) shipped from host, resident in SBUF for all 4 layers;
    applied MULTIPLICATIVELY on the Vector engine (at = exp(s) * expmask),
    so no PE matmuls or per-layer DMA for masking.
  - causal truncation: score/AV matmuls + exp only cover q >= kc*128 for
    key chunk kc (62.5% of full work).
  - Wo contracts K=128 over paired heads (ctx stored as [128, S] pairs).
  - All plain PSUM->SBUF copies/bias/residual ops run on Vector (DVE);
    Scalar engine only runs Exp / Gelu / Sqrt, avoiding activation-table
    thrashing.
  - token embeddings (val+ring gather) precomputed on host.
  - zero biases / unit LN scales detected host-side and elided.
"""

import os
import sys

for _p in ("/opt/trn_rl_repo",):
    if _p not in sys.path:
        sys.path.insert(0, _p)

import ml_dtypes
import numpy as np

import concourse.bass as bass
import concourse.mybir as mybir
import concourse.tile as tile
from concourse import bacc
from concourse.bass_utils import run_bass_kernel_spmd

BF16 = ml_dtypes.bfloat16

L, E, H, F = 4, 1024, 16, 4096
B, S = 8, 512
VV, VR = 40, 30
DIST_V = 200
PAD_ID = 0
DH = E // H  # 64
NE = E // 128  # 8 feature chunks
NO = 10  # logit row tiles (1280 padded)
EM_OFF = (0, 512, 896, 1152)  # causal-packed expmask offsets per kc
EM_W = 1280
FULLW = False

f32 = mybir.dt.float32
bf16 = mybir.dt.bfloat16
AF = mybir.ActivationFunctionType
OP = mybir.AluOpType

_CACHE = {}


# ----------------------------------------------------------------------------
# host-side input prep
# ----------------------------------------------------------------------------

def _b16(x):
    return np.ascontiguousarray(np.asarray(x).astype(BF16))


def _block_lhsT(W, gsize):
    # W: [L?, OUT, IN] -> [.., G, 128, IN//128, gsize] with
    # out[..., g, p, c, o] = W[..., g*gsize + o, c*128 + p]
    *lead, O, I = W.shape
    G = O // gsize
    nc_ = I // 128
    Wb = W.reshape(*lead, G, gsize, nc_, 128)
    Wb = np.moveaxis(Wb, -1, -3)  # [..., G, 128, gsize, nc]
    Wb = np.swapaxes(Wb, -1, -2)  # [..., G, 128, nc, gsize]
    return np.ascontiguousarray(Wb)


def _pp(v):  # [..., N*128] -> [..., 128, N]
    *lead, N = v.shape
    return np.ascontiguousarray(
        v.reshape(*lead, N // 128, 128).swapaxes(-1, -2).astype(np.float32)
    )


def _prep_shared(inp):
    """Weight-layout prep shared by all cores. Returns (tensors, flags)."""
    out = {}
    flags = {}

    Wqkv = np.asarray(inp["Wqkv"], np.float32).copy()  # [L, 3E, E]
    bqkv = np.asarray(inp["bqkv"], np.float32).copy()  # [L, 3E]
    # fold attention scale into Q projection
    scale = 1.0 / np.sqrt(DH)
    Wqkv[:, :E, :] *= scale
    bqkv[:, :E] *= scale

    out["wqkv"] = _b16(_block_lhsT(Wqkv, 512))          # [L, 6, 128, 8, 512]
    # negated input-dim sums of the (scaled) Q,K projections, for the
    # LN-deferred QKV fixup: qk = (ps - wsum*u) * rstd
    out["wqksum_row"] = _b16((-Wqkv[:, :2 * E, :].sum(-1)).reshape(L, 1, 2 * E))

    # Wo paired-head lhsT: wo[l, p, hp, mi, m] = Wo[l, mi*128+m, hp*128+p]
    Wo = np.asarray(inp["Wo"], np.float32)  # [L, E(out), E(in=ctx)]
    wo = Wo.reshape(L, 8, 128, 8, 128)      # [l, mi, m, hp, p]
    wo = wo.transpose(0, 4, 3, 1, 2)        # [l, p, hp, mi, m]
    out["wo"] = _b16(wo)                    # [L, 128, 8, 8, 128]

    out["w1"] = _b16(_block_lhsT(np.asarray(inp["W1"], np.float32), 512))
    W2 = np.asarray(inp["W2"], np.float32)  # out=E, in=F
    w2b = _block_lhsT(W2, 512)  # [L, 2, 128, 32, 512]
    w2b = w2b.reshape(L, 2, 128, 4, 8, 512).transpose(0, 1, 3, 2, 4, 5)
    out["w2"] = _b16(w2b)  # [L, 2, 4, 128, 8, 512]

    genW = np.asarray(inp["gen_W"], np.float32)  # [1200, E]
    genW_pad = np.zeros((1280, E), np.float32)
    genW_pad[:1200] = genW
    genw_b = _block_lhsT(genW_pad, 640).reshape(2, 128, 2, 4, 640)
    genw_b = genw_b.transpose(0, 2, 1, 3, 4).reshape(4, 128, 4, 640)
    out["genw"] = _b16(genw_b)  # [4, 128, 4, 640]

    # biases (elided when zero)
    flags["bqkv"] = bool(np.any(bqkv != 0))
    flags["bo"] = bool(np.any(np.asarray(inp["bo"]) != 0))
    flags["b1"] = bool(np.any(np.asarray(inp["b1"]) != 0))
    flags["b2"] = bool(np.any(np.asarray(inp["b2"]) != 0))
    flags["genb"] = bool(np.any(np.asarray(inp["gen_b"]) != 0))
    flags["ln_s"] = not bool(
        np.all(np.asarray(inp["ln1_s"]) == 1) and np.all(np.asarray(inp["ln2_s"]) == 1)
    )
    flags["ln_b"] = bool(
        np.any(np.asarray(inp["ln1_b"]) != 0) or np.any(np.asarray(inp["ln2_b"]) != 0)
    )
    flags["lnf_s"] = not bool(np.all(np.asarray(inp["lnf_s"]) == 1))
    flags["lnf_b"] = bool(np.any(np.asarray(inp["lnf_b"]) != 0))

    if flags["bqkv"]:
        out["bqkv_pp"] = _pp(bqkv[:, : 2 * E])  # [L, 128, 16] (Q scaled)
        out["bv_row"] = _b16(bqkv[:, 2 * E:].reshape(L, 1, E))  # [L, 1, E]
        out["ones_row"] = _b16(np.ones((1, S), np.float32))
    if flags["bo"]:
        out["bo_pp"] = _pp(np.asarray(inp["bo"], np.float32))  # [L, 128, 8]
    if flags["b1"]:
        out["b1_pp"] = _pp(np.asarray(inp["b1"], np.float32))  # [L, 128, 32]
    if flags["b2"]:
        out["b2_pp"] = _pp(np.asarray(inp["b2"], np.float32))  # [L, 128, 8]
    if flags["genb"]:
        gbp = np.zeros((1280,), np.float32)
        gbp[:1200] = np.asarray(inp["gen_b"], np.float32)
        out["gen_b_pp"] = np.ascontiguousarray(gbp.reshape(NO, 128).T)  # [128, 10]
    if flags["ln_s"]:
        ln_s = np.stack([np.asarray(inp["ln1_s"], np.float32),
                         np.asarray(inp["ln2_s"], np.float32)], 1)  # [L, 2, E]
        out["ln_s_pp"] = _pp(ln_s)  # [L, 2, 128, 8]
    if flags["ln_b"]:
        ln_b = np.stack([np.asarray(inp["ln1_b"], np.float32),
                         np.asarray(inp["ln2_b"], np.float32)], 1)
        out["ln_b_pp"] = _pp(ln_b)
    if flags["lnf_s"]:
        out["lnf_s_pp"] = _pp(np.asarray(inp["lnf_s"], np.float32))  # [128, 8]
    if flags["lnf_b"]:
        out["lnf_b_pp"] = _pp(np.asarray(inp["lnf_b"], np.float32))

    out["ones_col"] = _b16(np.ones((128, 1), np.float32))
    out["ones_r128"] = _b16(np.ones((1, 128), np.float32))
    return out, flags


def _prep_percore(inp):
    """Per-core tensors: embedded tokens + packed exp(mask)."""
    val = np.asarray(inp["val_sequences"]).astype(np.int64)    # [B, S]
    ring = np.asarray(inp["ring_sequences"]).astype(np.int64)  # [B, S]
    dist = np.asarray(inp["distance_squares"]).astype(np.int64)  # [B, S, S]
    de = np.asarray(inp["dist_emb"], np.float32)  # [200, H]
    ve = np.asarray(inp["val_emb"], np.float32)   # [VV, E]
    re = np.asarray(inp["ring_emb"], np.float32)  # [VR, E]

    # x0: feature-major embedded tokens [B, NE, 128, S]
    x0 = (ve[val] + re[ring]) * np.sqrt(E)                  # [B, S, E]
    x0 = x0.reshape(B, S, NE, 128).transpose(0, 2, 3, 1)    # [B, NE, 128, S]
    x0 = _b16(x0)

    # expmask, causal-packed: em[b, p, h*1280 + OFF[kc] + j]
    #   = exp(de[dist[b, qlo+j, kc*128+p]])  (0 where k>q or key is pad)
    g = np.exp(de[dist])                 # [B, q, k, H]
    kk = np.arange(S)
    keep = (kk[None, :] <= kk[:, None])  # [q, k]: k <= q
    g *= keep[None, :, :, None]
    g *= (val != PAD_ID)[:, None, :, None]
    emp = np.zeros((B, H, 128, EM_W), np.float32)
    for kc in range(4):
        qlo = kc * 128
        w = S - qlo
        blk = g[:, qlo:, kc * 128:(kc + 1) * 128, :]  # [B, w, 128, H]
        emp[:, :, :, EM_OFF[kc]:EM_OFF[kc] + w] = blk.transpose(0, 3, 2, 1)
    emp = _b16(emp.transpose(0, 2, 1, 3).reshape(B, 128, H * EM_W))

    cores = []
    for b in range(B):
        cores.append({"x0": np.ascontiguousarray(x0[b]),
                      "emask": np.ascontiguousarray(emp[b])})
    return cores


# ----------------------------------------------------------------------------
# device program
# ----------------------------------------------------------------------------

def _declare(nc, flags):
    d = {}

    def di(name, shape, dt):
        d[name] = nc.dram_tensor(name, list(shape), dt, kind="ExternalInput").ap()

    di("wqkv", (L, 6, 128, 8, 512), bf16)
    di("wqksum_row", (L, 1, 2 * E), bf16)
    di("wo", (L, 128, 8, 8, 128), bf16)
    di("w1", (L, 8, 128, 8, 512), bf16)
    di("w2", (L, 2, 4, 128, 8, 512), bf16)
    di("genw", (4, 128, 4, 640), bf16)
    if flags["bqkv"]:
        di("bqkv_pp", (L, 128, 16), f32)
        di("bv_row", (L, 1, E), bf16)
        di("ones_row", (1, S), bf16)
    if flags["bo"]:
        di("bo_pp", (L, 128, 8), f32)
    if flags["b1"]:
        di("b1_pp", (L, 128, 32), f32)
    if flags["b2"]:
        di("b2_pp", (L, 128, 8), f32)
    if flags["genb"]:
        di("gen_b_pp", (128, NO), f32)
    if flags["ln_s"]:
        di("ln_s_pp", (L, 2, 128, 8), f32)
    if flags["ln_b"]:
        di("ln_b_pp", (L, 2, 128, 8), f32)
    if flags["lnf_s"]:
        di("lnf_s_pp", (128, 8), f32)
    if flags["lnf_b"]:
        di("lnf_b_pp", (128, 8), f32)
    di("ones_col", (128, 1), bf16)
    di("ones_r128", (1, 128), bf16)
    di("x0", (NE, 128, S), bf16)
    di("emask", (128, H * EM_W), bf16)
    d["logits"] = nc.dram_tensor(
        "logits", [NO, 128, S], f32, kind="ExternalOutput"
    ).ap()
    if os.environ.get("BG_DEBUG"):
        def do(name, shape):
            d[name] = nc.dram_tensor(name, list(shape), bf16,
                                     kind="ExternalOutput").ap()
        do("dbg_h0", (NE, 128, S))
        do("dbg_qk", (16, 128, S))
        do("dbg_v", (4, 128, H, DH + 1))
        do("dbg_at", (8, 128, S))
        do("dbg_ctx", (8, 128, S))
        do("dbg_r1", (NE, 128, S))
        do("dbg_h1", (NE, 128, S))
        do("dbg_h2", (NE, 128, S))
    return d


def _emit(nc, tc, d, ctx, flags):
    mm = nc.tensor.matmul

    cpool = ctx.enter_context(tc.tile_pool(name="cpool", bufs=1))
    wpool = ctx.enter_context(tc.tile_pool(name="wpool", bufs=4))
    hpool = ctx.enter_context(tc.tile_pool(name="hpool", bufs=16))
    qkpool = ctx.enter_context(tc.tile_pool(name="qkpool", bufs=17))
    vpool = ctx.enter_context(tc.tile_pool(name="vpool", bufs=4))
    atpool = ctx.enter_context(tc.tile_pool(name="atpool", bufs=5))
    ctxpool = ctx.enter_context(tc.tile_pool(name="ctxpool", bufs=8))
    ffpool = ctx.enter_context(tc.tile_pool(name="ffpool", bufs=32))
    tmppool = ctx.enter_context(tc.tile_pool(name="tmppool", bufs=4))
    smallf = ctx.enter_context(tc.tile_pool(name="smallf", bufs=3))
    smallb = ctx.enter_context(tc.tile_pool(name="smallb", bufs=2))
    recpool = ctx.enter_context(tc.tile_pool(name="recpool", bufs=2))
    outpool = ctx.enter_context(tc.tile_pool(name="outpool", bufs=2))
    pppool = ctx.enter_context(tc.tile_pool(name="pppool", bufs=4))
    empool = ctx.enter_context(tc.tile_pool(name="empool", bufs=1))

    ps_gemm = ctx.enter_context(tc.tile_pool(name="ps_gemm", bufs=4, space="PSUM"))
    ps_ctx = ctx.enter_context(tc.tile_pool(name="ps_ctx", bufs=3, space="PSUM"))
    ps_ln = ctx.enter_context(tc.tile_pool(name="ps_ln", bufs=1, space="PSUM"))

    hw = nc.sync  # HWDGE dma engine

    # --- constants -----------------------------------------------------------
    ones_col = cpool.tile([128, 1], bf16)
    hw.dma_start(out=ones_col, in_=d["ones_col"])
    ones_r128 = cpool.tile([1, 128], bf16)
    hw.dma_start(out=ones_r128, in_=d["ones_r128"])
    eps_t = cpool.tile([128, 1], f32)
    nc.vector.memset(eps_t, 1e-5)
    genb_pp = None
    if flags["genb"]:
        genb_pp = cpool.tile([128, NO], f32)
        hw.dma_start(out=genb_pp, in_=d["gen_b_pp"])
    lnf_s = lnf_b = None
    if flags["lnf_s"]:
        lnf_s = cpool.tile([128, 8], f32)
        hw.dma_start(out=lnf_s, in_=d["lnf_s_pp"])
    if flags["lnf_b"]:
        lnf_b = cpool.tile([128, 8], f32)
        hw.dma_start(out=lnf_b, in_=d["lnf_b_pp"])
    ones_row = bv_all = None
    if flags["bqkv"]:
        ones_row = cpool.tile([1, S], bf16)
        hw.dma_start(out=ones_row, in_=d["ones_row"])

    # resident expmask (on SWDGE queue so it doesn't delay weight loads)
    em_sb = empool.tile([128, H * EM_W], bf16)
    nc.gpsimd.dma_start(out=em_sb, in_=d["emask"])

    # --- embedding (host-precomputed) ---------------------------------------
    h_t = []
    for c in range(NE):
        ht = hpool.tile([128, S], bf16, tag="h")
        hw.dma_start(out=ht, in_=d["x0"][c])
        if "dbg_h0" in d:
            nc.sync.dma_start(out=d["dbg_h0"][c], in_=ht)
        h_t.append(ht)

    env = dict(locals())
    env["ps_ln"] = ps_ln

    # --- layers --------------------------------------------------------------
    # When ln2 has unit scale / zero bias, its output is exactly normalized
    # (mean 0, var 1-eps'), so a unit-scale/zero-bias final LN is an identity
    # up to O(eps) -- skip it.
    skip_lnf = not (flags["lnf_s"] or flags["lnf_b"]
                    or flags["ln_s"] or flags["ln_b"])
    lnacc_f = [None]
    qkv_src = None
    for l in range(L):
        h_t, qkv_src = _layer(
            nc, tc, d, l, h_t, env, flags, qkv_src=qkv_src,
            lnacc_f=lnacc_f if (l == L - 1 and not skip_lnf) else None)

    # --- final LN + head -----------------------------------------------------
    with nc.named_scope("final"):
        if skip_lnf:
            hf = h_t
        else:
            rstdR, uR = lnacc_f[0].stats()
            hf = _ln_normalize(nc, env, h_t, rstdR, uR, lnf_s, lnf_b)
        genw_sb = env["genw_sb"]
        for mt in range(NO):
            g, mi = divmod(mt, 5)
            ps = ps_gemm.tile([128, S], f32, tag="gemm")
            for c in range(NE):
                mm(ps, genw_sb[g * 2 + c // 4][:, c % 4, mi * 128:(mi + 1) * 128],
                   hf[c],
                   start=(c == 0), stop=(c == NE - 1))
            ot = outpool.tile([128, S], f32, tag="f32out")
            if flags["genb"]:
                nc.vector.tensor_scalar(ot, ps, genb_pp[:, mt:mt + 1], None, OP.add)
            else:
                nc.vector.tensor_copy(ot, ps)
            hw.dma_start(out=d["logits"][mt], in_=ot)


class _LNAcc:
    """Pipelined LN statistics: sum/sumsq matmuls emitted as chunks land."""

    def __init__(self, nc, env, name, pool=None, tag="ln"):
        self.nc = nc
        self.env = env
        self.name = name
        pool = pool if pool is not None else env["ps_ln"]
        sums = pool.tile([33, S], f32, tag=tag, name=f"lns_{name}")
        self.sums_r = sums[0:1, :]
        self.sums_q = sums[32:33, :]

    def add(self, c, r_tile):
        nc, env = self.nc, self.env
        sq = env["tmppool"].tile([128, S], bf16, tag="sq",
                                 name=f"sq_{self.name}_{c}")
        nc.scalar.activation(sq, r_tile, AF.Square)
        mm = nc.tensor.matmul
        mm(self.sums_r, env["ones_col"], r_tile, start=(c == 0),
           stop=(c == NE - 1))
        mm(self.sums_q, env["ones_col"], sq, start=(c == 0),
           stop=(c == NE - 1))

    def stats(self, sbuf_bcast=False):
        nc, env = self.nc, self.env
        smallf = env["smallf"]; smallb = env["smallb"]; recpool = env["recpool"]
        nm = self.name
        s2 = smallf.tile([1, S], f32, tag="sf", name=f"s2_{nm}")
        nc.scalar.activation(s2, self.sums_r, AF.Square)
        varE = smallf.tile([1, S], f32, tag="sf", name=f"ve_{nm}")
        # varE = sumsq - s2/E  (= E * var)
        nc.vector.scalar_tensor_tensor(varE, s2, -1.0 / E, self.sums_q,
                                       OP.mult, OP.add)
        rstd = smallf.tile([1, S], f32, tag="sf", name=f"rs_{nm}")
        nc.scalar.activation(rstd, varE, AF.Abs_reciprocal_sqrt,
                             bias=env["eps_t"][:1, :], scale=1.0 / E)
        ru = smallb.tile([1, 3 * S], bf16, tag="sb", name=f"ru_{nm}")
        nc.vector.tensor_copy(ru[:, 0:S], rstd)
        # u*rstd = (sum/E) * rstd
        nc.vector.scalar_tensor_tensor(ru[:, S:2 * S], self.sums_r, 1.0 / E,
                                       rstd, OP.mult, OP.mult)
        if sbuf_bcast:
            # plain u row for the K=1 fixup matmul
            nc.vector.tensor_scalar(ru[:, 2 * S:3 * S], self.sums_r, 1.0 / E,
                                    None, OP.mult)
            ruR = recpool.tile([128, 2 * S], bf16, tag="rec", name=f"ruR_{nm}")
            nc.gpsimd.partition_broadcast(ruR, ru[:, 0:2 * S], channels=128)
            return ruR, ru[:, 2 * S:3 * S]
        mm = nc.tensor.matmul
        ps_ctx = env["ps_ctx"]
        rstdR = ps_ctx.tile([128, S], f32, tag="ctxps", name=f"bcr_{nm}")
        mm(rstdR, env["ones_r128"], ru[:, 0:S], start=True, stop=True)
        uR = ps_ctx.tile([128, S], f32, tag="ctxps", name=f"bcu_{nm}")
        mm(uR, env["ones_r128"], ru[:, S:2 * S], start=True, stop=True)
        return rstdR, uR


def _ln_normalize_defer(nc, env, r_t, ruR):
    # h = r*rstd - u*rstd, all-SBUF bf16 ops
    out_t = []
    for c in range(NE):
        t2 = env["tmppool"].tile([128, S], bf16, tag="tmp")
        nc.vector.tensor_mul(t2, r_t[c], ruR[:, 0:S])
        ht = env["hpool"].tile([128, S], bf16, tag="h")
        nc.vector.tensor_sub(ht, t2, ruR[:, S:2 * S])
        out_t.append(ht)
    return out_t


def _ln_normalize(nc, env, r_t, rstdR, uR, s_pp, b_pp, nxt=None):
    out_t = []
    for c in range(NE):
        t2 = env["tmppool"].tile([128, S], bf16, tag="tmp")
        nc.vector.tensor_sub(t2, r_t[c], uR)
        ht = env["hpool"].tile([128, S], bf16, tag="h")
        sc = s_pp[:, c:c + 1] if s_pp is not None else 1.0
        nc.vector.scalar_tensor_tensor(ht, t2, sc, rstdR, OP.mult, OP.mult)
        if b_pp is not None:
            nc.vector.tensor_scalar(ht, ht, b_pp[:, c:c + 1], None, OP.add)
        if nxt is not None:
            nxt.add(c, ht)
        out_t.append(ht)
    return out_t


def _layer(nc, tc, d, l, h_t, env, flags, qkv_src=None, lnacc_f=None):
    mm = nc.tensor.matmul
    hw = nc.sync
    wpool = env["wpool"]; hpool = env["hpool"]; qkpool = env["qkpool"]
    vpool = env["vpool"]; atpool = env["atpool"]
    ctxpool = env["ctxpool"]; ffpool = env["ffpool"]; tmppool = env["tmppool"]
    smallf = env["smallf"]; smallb = env["smallb"]; recpool = env["recpool"]
    pppool = env["pppool"]
    ps_gemm = env["ps_gemm"]; ps_ctx = env["ps_ctx"]
    ones_col = env["ones_col"]; em_sb = env["em_sb"]

    # per-layer small params
    bqkv_pp = bv_row = bo_pp = b1_pp = b2_pp = None
    ln_s = [None, None]
    ln_b = [None, None]
    if flags["bqkv"]:
        bqkv_pp = pppool.tile([128, 16], f32, tag="pp16")
        hw.dma_start(out=bqkv_pp, in_=d["bqkv_pp"][l])
        bv_row = pppool.tile([1, E], bf16, tag="bvrow", bufs=2)
        hw.dma_start(out=bv_row, in_=d["bv_row"][l])
    if flags["bo"]:
        bo_pp = pppool.tile([128, 8], f32, tag="pp8")
        hw.dma_start(out=bo_pp, in_=d["bo_pp"][l])
    if flags["b1"]:
        b1_pp = pppool.tile([128, 32], f32, tag="pp32")
        hw.dma_start(out=b1_pp, in_=d["b1_pp"][l])
    if flags["b2"]:
        b2_pp = pppool.tile([128, 8], f32, tag="pp8")
        hw.dma_start(out=b2_pp, in_=d["b2_pp"][l])
    if flags["ln_s"]:
        ln_s = [pppool.tile([128, 8], f32, tag="pp8", name=f"lns{l}_{i}")
                for i in range(2)]
        for i in range(2):
            hw.dma_start(out=ln_s[i], in_=d["ln_s_pp"][l, i])
    if flags["ln_b"]:
        ln_b = [pppool.tile([128, 8], f32, tag="pp8", name=f"lnb{l}_{i}")
                for i in range(2)]
        for i in range(2):
            hw.dma_start(out=ln_b[i], in_=d["ln_b_pp"][l, i])

    # --- QKV -----------------------------------------------------------------
    with nc.named_scope(f"L{l}_qkv"):
        qk_t = []  # 16 tiles: q 0..7 (2 heads each), k 8..15
        if qkv_src is not None:
            r2_prev, ruR_prev, u_row, negw_row = qkv_src
            qkv_rhs = r2_prev
        else:
            qkv_rhs = h_t
        for g in range(4):  # Q, K feature-major
            wt = wpool.tile([128, 8, 512], bf16, tag="w")
            hw.dma_start(out=wt, in_=d["wqkv"][l, g])
            for mi in range(4):
                mt = g * 4 + mi
                ps = ps_gemm.tile([128, S], f32, tag="gemm")
                for c in range(NE):
                    last = (c == NE - 1) and qkv_src is None
                    mm(ps, wt[:, c, mi * 128:(mi + 1) * 128], qkv_rhs[c],
                       start=(c == 0), stop=last)
                if qkv_src is not None:
                    # LN-deferred fixup part 1: ps -= wsum * u  (K=1 matmul)
                    mm(ps, negw_row[:, mt * 128:(mt + 1) * 128], u_row,
                       start=False, stop=True)
                qk = qkpool.tile([128, S], bf16, tag="qk")
                if qkv_src is not None:
                    # part 2: qk = ps * rstd
                    nc.vector.tensor_mul(qk, ps, ruR_prev[:, 0:S])
                    if flags["bqkv"]:
                        nc.vector.tensor_scalar(qk, qk, bqkv_pp[:, mt:mt + 1],
                                                None, OP.add)
                elif flags["bqkv"]:
                    nc.scalar.activation(qk, ps, AF.Identity,
                                         bias=bqkv_pp[:, mt:mt + 1])
                else:
                    nc.scalar.activation(qk, ps, AF.Copy)
                if l == 0 and "dbg_qk" in d:
                    hw.dma_start(out=d["dbg_qk"][mt], in_=qk)
                qk_t.append(qk)
        if qkv_src is not None:
            # materialize h2 of the previous layer (residual/V input)
            h_t = _ln_normalize_defer(nc, env, r2_prev, ruR_prev)
        # V token-major, augmented with ones column
        v_t = []
        for n in range(4):
            vt = vpool.tile([128, H, DH + 1], bf16, tag="v")
            nc.vector.memset(vt[:, :, DH:DH + 1], 1.0)
            v_t.append(vt)
        for g in range(2):
            wt = wpool.tile([128, 8, 512], bf16, tag="w")
            hw.dma_start(out=wt, in_=d["wqkv"][l, 4 + g])
            for n in range(4):
                ps = ps_gemm.tile([128, S], f32, tag="gemm")
                for c in range(NE):
                    last = (c == NE - 1) and not flags["bqkv"]
                    mm(ps, h_t[c][:, n * 128:(n + 1) * 128], wt[:, c, :],
                       start=(c == 0), stop=last)
                if flags["bqkv"]:
                    mm(ps, env["ones_row"][:, :128],
                       bv_row[:, g * 512:(g + 1) * 512],
                       start=False, stop=True)
                nc.scalar.activation(
                    v_t[n][:, g * 8:(g + 1) * 8, 0:DH],
                    ps.rearrange("p (a b) -> p a b", a=8), AF.Copy)

    if l == 0 and "dbg_v" in d:
        for n in range(4):
            hw.dma_start(out=d["dbg_v"][n], in_=v_t[n])

    # --- attention ------------------------------------------------------------
    with nc.named_scope(f"L{l}_attn"):
        wo_ts = []
        for wh in range(2):
            wt = wpool.tile([128, 4, 8, 128], bf16, tag="w", name=f"wo{l}_{wh}")
            hw.dma_start(out=wt, in_=d["wo"][l][:, wh * 4:(wh + 1) * 4])
            wo_ts.append(wt)
        ctxp = [ctxpool.tile([128, S], bf16, tag="ctx", name=f"cp{l}_{i}")
                for i in range(8)]
        at_q = {}

        def emit_scores(h):
            qt = qk_t[h // 2]
            kt = qk_t[8 + h // 2]
            r0 = (h % 2) * DH
            ath = atpool.tile([128, EM_W], bf16, tag="at", name=f"a{l}_{h}")
            for kc in range(4):
                qlo = kc * 128
                N = S - qlo
                sps = ps_gemm.tile([128, S], f32, tag="gemm", name=f"s{l}_{h}_{kc}")
                mm(sps[:, :N], kt[r0:r0 + DH, kc * 128:(kc + 1) * 128],
                   qt[r0:r0 + DH, qlo:S], start=True, stop=True)
                nc.scalar.activation(ath[:, EM_OFF[kc]:EM_OFF[kc] + N],
                                     sps[:, :N], AF.Exp)
            nc.vector.tensor_mul(ath, ath,
                                 em_sb[:, h * EM_W:(h + 1) * EM_W])
            at_q[h] = ath

        def emit_av(h):
            ath = at_q.pop(h)
            cps = ps_ctx.tile([DH + 1, S], f32, tag="ctxps", name=f"c{l}_{h}")
            for kc in range(4):
                qlo = kc * 128
                N = S - qlo
                mm(cps[:, qlo:S], v_t[kc][:, h, :],
                   ath[:, EM_OFF[kc]:EM_OFF[kc] + N],
                   start=(kc == 0), stop=(kc == 3), skip_group_check=True)
            srow = smallf.tile([1, S], f32, tag="sf", name=f"sr{l}_{h}")
            nc.scalar.activation(srow, cps[DH:DH + 1, :], AF.Copy)
            rec = smallf.tile([1, S], f32, tag="sf", name=f"re{l}_{h}")
            nc.vector.reciprocal_approx_fast(out=rec, in_=srow)
            recR = recpool.tile([DH, S], f32, tag="rec", name=f"rr{l}_{h}")
            nc.gpsimd.partition_broadcast(recR, rec, channels=DH)
            hp = h // 2
            if h % 2 == 0:
                nc.vector.tensor_mul(ctxp[hp][0:DH, :], cps[0:DH, :], recR)
            else:
                chh = tmppool.tile([DH, S], bf16, tag="ate", name=f"ch{l}_{h}")
                nc.vector.tensor_mul(chh, cps[0:DH, :], recR)
                nc.vector.tensor_copy(ctxp[hp][DH:128, :], chh)

        emit_scores(0)
        emit_scores(1)
        emit_scores(2)
        for h in range(3, H):
            emit_scores(h)
            emit_av(h - 3)
        scrap = smallf.tile([1, 1], f32, tag="scrap", bufs=2, name=f"scr{l}a")
        nc.scalar.activation(scrap, at_q[H - 1][:1, :1], AF.Abs_reciprocal_sqrt)
        emit_av(H - 3)
        emit_av(H - 2)
        emit_av(H - 1)

        if l == 0 and "dbg_at" in d:
            pass  # at tiles are popped; skip
        if l == 0 and "dbg_ctx" in d:
            for i in range(8):
                hw.dma_start(out=d["dbg_ctx"][i], in_=ctxp[i])

        # out-proj (paired heads, K=128) + residual
        r1_t = []
        lnacc1 = _LNAcc(nc, env, f"l{l}a")
        for wave in range(2):
            pss = [ps_gemm.tile([128, S], f32, tag="gemm",
                                name=f"wops{l}_{wave}_{i}") for i in range(4)]
            for hp in range(8):
                for i in range(4):
                    mm(pss[i], wo_ts[hp // 4][:, hp % 4, wave * 4 + i, :],
                       ctxp[hp], start=(hp == 0), stop=(hp == 7))
            for i in range(4):
                mi = wave * 4 + i
                r1 = hpool.tile([128, S], bf16, tag="h", name=f"r1_{l}_{mi}")
                bsc = bo_pp[:, mi:mi + 1] if flags["bo"] else 0.0
                nc.vector.scalar_tensor_tensor(r1, pss[i], bsc, h_t[mi],
                                               OP.add, OP.add)
                lnacc1.add(mi, r1)
                if l == 0 and "dbg_r1" in d:
                    hw.dma_start(out=d["dbg_r1"][mi], in_=r1)
                r1_t.append(r1)

    with nc.named_scope(f"L{l}_ln1"):
        rstdR, uR = lnacc1.stats()
        h1_t = _ln_normalize(nc, env, r1_t, rstdR, uR, ln_s[0], ln_b[0])
        if l == 0 and "dbg_h1" in d:
            for c in range(NE):
                hw.dma_start(out=d["dbg_h1"][c], in_=h1_t[c])

    # --- FFN -----------------------------------------------------------------
    with nc.named_scope(f"L{l}_ffn"):
        scrapg = smallf.tile([1, 1], f32, tag="scrap", bufs=2, name=f"scr{l}g")
        nc.scalar.activation(scrapg, h1_t[0][:1, :1], AF.Gelu)
        ff_t = []
        for g in range(8):
            wt = wpool.tile([128, 8, 512], bf16, tag="w")
            hw.dma_start(out=wt, in_=d["w1"][l, g])
            for mi in range(4):
                mt = g * 4 + mi
                ps = ps_gemm.tile([128, S], f32, tag="gemm")
                for c in range(NE):
                    mm(ps, wt[:, c, mi * 128:(mi + 1) * 128], h1_t[c],
                       start=(c == 0), stop=(c == NE - 1))
                ft = ffpool.tile([128, S], bf16, tag="ff")
                if flags["b1"]:
                    nc.scalar.activation(ft, ps, AF.Gelu,
                                         bias=b1_pp[:, mt:mt + 1])
                else:
                    nc.scalar.activation(ft, ps, AF.Gelu)
                ff_t.append(ft)
        scrap2 = smallf.tile([1, 1], f32, tag="scrap", bufs=2, name=f"scr{l}b")
        nc.scalar.activation(scrap2, ff_t[31][:1, :1], AF.Abs_reciprocal_sqrt)
        r2_t = [None] * NE
        lnacc2 = _LNAcc(nc, env, f"l{l}f")
        for half in range(2):
            pss = [ps_gemm.tile([128, S], f32, tag="gemm",
                                name=f"ff2ps{l}_{half}_{i}") for i in range(4)]
            for cg in range(4):
                wt = wpool.tile([128, 8, 512], bf16, tag="w")
                hw.dma_start(out=wt, in_=d["w2"][l, half, cg])
                for c8 in range(8):
                    c = cg * 8 + c8
                    for mi in range(4):
                        mm(pss[mi], wt[:, c8, mi * 128:(mi + 1) * 128], ff_t[c],
                           start=(c == 0), stop=(c == 31))
            for mi in range(4):
                mt = half * 4 + mi
                r2 = hpool.tile([128, S], bf16, tag="h")
                bsc = b2_pp[:, mt:mt + 1] if flags["b2"] else 0.0
                nc.vector.scalar_tensor_tensor(r2, pss[mi], bsc, h1_t[mt],
                                               OP.add, OP.add)
                lnacc2.add(mt, r2)
                r2_t[mt] = r2

    if l == L - 1:
        genw_sb = []
        for gi in range(4):
            wt = wpool.tile([128, 4, 640], bf16, tag="w", name=f"genw{gi}")
            hw.dma_start(out=wt, in_=d["genw"][gi])
            genw_sb.append(wt)
        env["genw_sb"] = genw_sb

    with nc.named_scope(f"L{l}_ln2"):
        defer = (l < L - 1) and not (flags["ln_s"] or flags["ln_b"])
        if defer:
            ruR, u_row = lnacc2.stats(sbuf_bcast=True)
            scrap3 = smallf.tile([1, 1], f32, tag="scrap", bufs=2,
                                 name=f"scr{l}c")
            nc.scalar.activation(scrap3, ruR[:1, :1], AF.Exp)
            negw_row = pppool.tile([1, 2 * E], bf16, tag="negw", bufs=2,
                                   name=f"negw{l + 1}")
            hw.dma_start(out=negw_row, in_=d["wqksum_row"][l + 1])
            return None, (r2_t, ruR, u_row, negw_row)
        rstdR, uR = lnacc2.stats()
        if l < L - 1:
            scrap3 = smallf.tile([1, 1], f32, tag="scrap", bufs=2,
                                 name=f"scr{l}c")
            nc.scalar.activation(scrap3, rstdR[:1, :1], AF.Exp)
        nxt = None
        if lnacc_f is not None:
            lnacc_f[0] = _LNAcc(nc, env, "f", pool=env["ps_gemm"], tag="gemm")
            nxt = lnacc_f[0]
        h2_t = _ln_normalize(nc, env, r2_t, rstdR, uR, ln_s[1], ln_b[1],
                             nxt=nxt)
        if l == 0 and "dbg_h2" in d:
            for c in range(NE):
                hw.dma_start(out=d["dbg_h2"][c], in_=h2_t[c])
    return h2_t, None


def _build(flags):
    key = tuple(sorted(flags.items()))
    if key in _CACHE:
        return _CACHE[key]
    from contextlib import ExitStack

    nc = bacc.Bacc("TRN2", debug=False)
    d = _declare(nc, flags)
    with tile.TileContext(nc) as tc:
        with ExitStack() as ctx:
            _emit(nc, tc, d, ctx, flags)
    nc.compile()
    _CACHE[key] = nc
    return nc


def kernel_internal(inputs, trace=False, trace_kwargs=None):
    shared, flags = _prep_shared(inputs)
    cores = _prep_percore(inputs)
    nc = _build(flags)
    in_maps = []
    for b in range(B):
        m = dict(shared)
        m.update(cores[b])
        in_maps.append(m)
    res = run_bass_kernel_spmd(
        nc, in_maps, core_ids=list(range(B)), trace=trace,
        **(trace_kwargs or {}),
    )
    outs = []
    for b in range(B):
        lo = res.results[b]["logits"]  # [10, 128, 512]
        lo = lo.reshape(NO * 128, S)[:VV * VR].T  # [512, 1200]
        outs.append(lo)
    out = np.stack(outs).astype(np.float32)  # [B, S, 1200]
    return out, res


def kernel(**inputs):
    out, _ = kernel_internal(inputs)
    return out
